# revision 2
# baseline (speedup 1.0000x reference)
"""Trainium2 Bass kernel: GQA sliding-window attention (bf16 redesign).

Problem: B=1, T=4096, D=2048, H=16 q-heads, KVH=4 kv-heads, HD=128,
causal sliding window 512.

Sharding: 8-way sequence parallel. Core c owns query rows
[512c, 512c+512). It receives xT columns for rows [512(c-1), 512(c+1))
(halo of 512 rows; core 0's halo is zeros and is masked out). Weights
replicated. Outputs are disjoint row blocks -> concatenation.

All compute in bf16 (f32 PSUM accumulation). Host pre-transposes x
(so no on-device transpose phase), pre-scales Wq by SCALE, and casts
everything to bf16.

Per-core layouts (SBUF partition dim first):
  xT  [128, 16, 1024] bf16 : xT[p, dc, j] = x[j, 128*dc+p]
  kT  [128, 4, 1024]  bf16 : kT[p, g, j]  = k[j, 128*g+p]
  vv  [128, 8, 512]   bf16 : vv[p, jc, e] = v[128*jc+p, e]
  qT  [128, 16, 512]  bf16 : qT[p, h, i]  = q[i, 128*h+p] (SCALE folded
    into Wq host-side)
  Scores per (h, t): s[i', jj], key j = 128*t + jj, jj in [0,640).
  Softmax without max-subtraction (|scores| < ~6 for this input
    distribution, verified host-side in the test harness).
  Multiplicative {0,1} bf16 mask post-exp, fused with the row-sum
    (scalar_tensor_tensor accum_out).
  Normalization: w *= 1/rowsum on DVE (bf16 4x mode) before the PE
    transpose. (The PE transpose rhs must be a permutation matrix, so
    normalization cannot fold into it.)
  PV per (h, t): 5 chunk matmuls over the 640-key band into a
    [128, 128] window of the per-head po accumulator.
  oT  [128, 16, 512] bf16 ; y = oT.T @ Wo streamed in 512-col blocks.

P1 is paced against the xT DMA chunks: k-projections for g0/g1/g2 keep
6 accumulation groups open (4 ps_s half-windows + 2 ps_ot) and consume
each 4-dc xT chunk as it lands; g3 + v run afterwards from SBUF.
"""

import numpy as np

T = 4096
D = 2048
H = 16
KVH = 4
HD = 128
WINDOW = 512
SCALE = HD ** -0.5
N_CORES = 8
TLOC = T // N_CORES          # 512 own query rows / core
XROWS = TLOC + WINDOW        # 1024 x rows / core (halo + own)
NT = TLOC // 128             # 4 q-tiles of 128 rows
NJC = XROWS // 128           # 8 key chunks of 128
BAND = WINDOW + 128          # 640 key columns per q-tile
DC = D // 128                # 16 d-chunks

_CACHE = {}


def _emit(nc, tc, tile, mybir, make_identity, loop_n=None):
    f32 = mybir.dt.float32
    bf16 = mybir.dt.bfloat16

    timing = loop_n is not None
    kin = "Internal" if timing else "ExternalInput"
    kout = "Internal" if timing else "ExternalOutput"
    # xt: x transposed host-side, [(dc p), j] = x[j, 128*dc+p]
    xt_d = nc.dram_tensor("xt", [D, XROWS], bf16, kind=kin)
    wq_d = nc.dram_tensor("wq", [D, H * HD], bf16, kind=kin)
    wk_d = nc.dram_tensor("wk", [D, KVH * HD], bf16, kind=kin)
    wv_d = nc.dram_tensor("wv", [D, KVH * HD], bf16, kind=kin)
    wo_d = nc.dram_tensor("wo", [H * HD, D], bf16, kind=kin)
    mask_d = nc.dram_tensor("mask", [NT, 128, BAND], bf16, kind=kin)
    y_d = nc.dram_tensor("y", [TLOC, D], bf16, kind=kout)
    if timing:
        dummy_d = nc.dram_tensor("bench_done", [1, 128], f32,
                                 kind="ExternalOutput")

    def mm(out, lhsT, rhs, start, stop):
        nc.tensor.matmul(out, lhsT, rhs, start=start, stop=stop)

    # --- persistent pools (outside timing loop) ---
    # PSUM budget (8 banks): ps_s = 2 bufs x [128,1024]f32 (2 banks each)
    # = 4 banks; ps_pt = 2 bufs x [128,640]bf16 (1 bank each) = 2 banks;
    # ps_ot = 2 bufs x [128,512]f32 = 2 banks.
    pers = tc.alloc_tile_pool(name="pers", bufs=1)
    ps_s = tc.alloc_tile_pool(name="ps_s", bufs=2, space="PSUM")
    ps_pt = tc.alloc_tile_pool(name="ps_pt", bufs=2, space="PSUM")
    ps_ot = tc.alloc_tile_pool(name="ps_ot", bufs=2, space="PSUM")

    identb = pers.tile([128, 128], bf16, tag="identb")
    make_identity(nc, identb[:])

    lp = tc.For_i(0, loop_n, 1) if timing else None
    if lp is not None:
        lp.__enter__()

    proj = tc.alloc_tile_pool(name="proj", bufs=1)
    mask_s = proj.tile([128, NT, BAND], bf16, tag="mask")
    qT = proj.tile([128, H, TLOC], bf16, tag="qT")
    kT = proj.tile([128, KVH, XROWS], bf16, tag="kT")
    vv = proj.tile([128, NJC, KVH * HD], bf16, tag="vv")
    oT = proj.tile([128, H, TLOC], bf16, tag="oT")
    xtp = tc.alloc_tile_pool(name="xtp", bufs=1)
    xT = xtp.tile([128, DC, XROWS], bf16, tag="xT")

    # weight pools (right side; kvw above wp so kvw frees after P1)
    wp = tc.alloc_tile_pool(name="wp", bufs=4, side="right")
    kvw = tc.alloc_tile_pool(name="kvw", bufs=1, side="right")

    # ---------------- DMA issue (sync queue, in need-order) -------------
    # Order paces P1: wk(g0,g1) -> xt chunks interleaved with wk(g2,g3).
    wk_s = kvw.tile([128, DC, KVH * HD], bf16, tag="wk")
    wv_s = kvw.tile([128, DC, KVH * HD], bf16, tag="wv")
    nc.sync.dma_start(
        wk_s[:, :, 0:256],
        wk_d.ap()[:, 0:256].rearrange("(c p) e -> p c e", p=128))
    # first two xT chunks are 2-dc so PE can start sooner
    for lo, hi in ((0, 2), (2, 4), (4, 8), (8, 12), (12, 16)):
        nc.sync.dma_start(
            xT[:, lo:hi, :],
            xt_d.ap()[lo * 128:hi * 128, :]
            .rearrange("(c p) j -> p c j", p=128))
        if hi == 4:
            nc.sync.dma_start(
                wk_s[:, :, 256:512],
                wk_d.ap()[:, 256:512].rearrange("(c p) e -> p c e", p=128))
    nc.sync.dma_start(
        wv_s[:], wv_d.ap().rearrange("(c p) e -> p c e", p=128))
    wq_c = []
    for cq in range(4):
        wqc = wp.tile([128, DC, 512], bf16, tag="wq", name=f"wq{cq}")
        nc.sync.dma_start(
            wqc[:],
            wq_d.ap()[:, cq * 512:(cq + 1) * 512]
            .rearrange("(c p) e -> p c e", p=128))
        wq_c.append(wqc)
        if cq == 0:
            nc.sync.dma_start(mask_s[:],
                              mask_d.ap().rearrange("t p j -> p t j"))

    # ---------------- P1: k/v projections ----------------
    # P1a: k for g0..g2, 6 open accumulation groups, paced by xT chunks.
    sA = ps_s.tile([128, 1024], f32, tag="score", name="p1_sA")
    sB = ps_s.tile([128, 1024], f32, tag="score", name="p1_sB")
    oA = ps_ot.tile([128, 512], f32, tag="ot", name="p1_oA")
    oB = ps_ot.tile([128, 512], f32, tag="ot", name="p1_oB")
    kacc = {  # (g, half) -> psum window
        (0, 0): sA[:, 0:512], (0, 1): sA[:, 512:1024],
        (1, 0): sB[:, 0:512], (1, 1): sB[:, 512:1024],
        (2, 0): oA[:], (2, 1): oB[:],
    }
    for lo, hi in ((0, 2), (2, 4), (4, 8), (8, 12), (12, 16)):
        for g in range(3):
            for half in range(2):
                for dc in range(lo, hi):
                    mm(kacc[(g, half)],
                       wk_s[:, dc, g * 128:(g + 1) * 128],
                       xT[:, dc, half * 512:(half + 1) * 512],
                       start=(dc == 0), stop=(dc == DC - 1))
    for i, ((g, half), acc) in enumerate(kacc.items()):
        if i % 2 == 0:
            nc.vector.tensor_copy(
                kT[:, g, half * 512:(half + 1) * 512], acc)
        else:
            nc.scalar.copy(kT[:, g, half * 512:(half + 1) * 512], acc)

    # P1b: k g3 + v, xT fully resident.
    for half in range(2):
        pk = ps_ot.tile([128, 512], f32, tag="ot", name=f"pk3_{half}")
        for dc in range(DC):
            mm(pk[:], wk_s[:, dc, 384:512],
               xT[:, dc, half * 512:(half + 1) * 512],
               start=(dc == 0), stop=(dc == DC - 1))
        if half == 0:
            nc.vector.tensor_copy(kT[:, 3, 0:512], pk[:])
        else:
            nc.scalar.copy(kT[:, 3, 512:1024], pk[:])

    for jc in range(NJC):
        pv = ps_ot.tile([128, 512], f32, tag="ot", name=f"pv{jc}")
        for dc in range(DC):
            mm(pv[:], xT[:, dc, jc * 128:(jc + 1) * 128], wv_s[:, dc, :],
               start=(dc == 0), stop=(dc == DC - 1))
        if jc % 2 == 0:
            nc.vector.tensor_copy(vv[:, jc, :], pv[:])
        else:
            nc.scalar.copy(vv[:, jc, :], pv[:])

    kvw.release()

    # ---------------- P2: attention, q projection interleaved ----------
    sm = tc.alloc_tile_pool(name="sm", bufs=2, side="right")

    def qproj(h):
        """q projection for head h (runs one head ahead of scores)."""
        wqc = wq_c[h // 4]
        e0 = (h % 4) * 128
        pq = ps_ot.tile([128, TLOC], f32, tag="ot", name=f"pq{h}")
        for dc in range(DC):
            mm(pq[:], wqc[:, dc, e0:e0 + 128], xT[:, dc, WINDOW:XROWS],
               start=(dc == 0), stop=(dc == DC - 1))
        nc.scalar.copy(qT[:, h, :], pq[:])

    def iter_body(h):
        """Emit one pipeline iteration.

        h: head whose scores/softmax run this iteration (None past end).
        Returns state for the next iteration.
        """
        hp = h - 1 if h is not None else H - 1  # stage_b head
        do_b = hp >= 0 and iter_body.prev is not None
        if do_b:
            wt_list = iter_body.prev
            gb = hp // (H // KVH)
            wT_list = []
        cur = None
        if h is not None:
            g = h // (H // KVH)
            lsum4 = sm.tile([128, NT], f32, tag="l4", name=f"l4_{h}",
                            bufs=2)
            r4 = sm.tile([128, NT], f32, tag="r4", name=f"r4_{h}", bufs=2)
            wt_new = []

        # interleave: transposes(hp, t) with scores(h, t). On the last
        # scores head, emit scores before transposes so the softmax
        # chain of h=15 starts early (shortens the pipeline drain).
        order = ("b", "a") if h != H - 1 else ("a", "b")
        for t in range(NT):
          for phase in order:
            if phase == "b" and do_b:
                pt = ps_pt.tile([128, BAND], bf16, tag="pt",
                                name=f"pt{hp}_{t}")
                for b in range(5):
                    nc.tensor.transpose(
                        pt[:, b * 128:(b + 1) * 128],
                        wt_list[t][:, b * 128:(b + 1) * 128],
                        identb[:])
                wT = sm.tile([128, BAND], bf16, tag="wT",
                             name=f"wT{hp}_{t}", bufs=3)
                nc.vector.tensor_copy(wT[:], pt[:])
                wT_list.append(wT)
                if h is None:
                    # drain iteration: no scores/qproj work to hide the
                    # copy latency, so run each PV right after its tile
                    if t == 0:
                        po_d = ps_ot.tile([128, TLOC], f32, tag="ot",
                                          name=f"po{hp}")
                    for b in range(5):
                        mm(po_d[:, t * 128:(t + 1) * 128],
                           vv[:, t + b, gb * 128:(gb + 1) * 128],
                           wT[:, b * 128:(b + 1) * 128],
                           start=(b == 0), stop=(b == 4))
                    if t == NT - 1:
                        nc.scalar.copy(oT[:, hp, :], po_d[:])
            if phase == "a" and h is not None:
                ps = ps_s.tile([128, 1024], f32, tag="score",
                               name=f"s{h}_{t}")
                mm(ps[:, 0:512], qT[:, h, t * 128:(t + 1) * 128],
                   kT[:, g, t * 128:t * 128 + 512], start=True, stop=True)
                mm(ps[:, 512:BAND], qT[:, h, t * 128:(t + 1) * 128],
                   kT[:, g, t * 128 + 512:t * 128 + BAND], start=True,
                   stop=True)
                w = sm.tile([128, BAND], bf16, tag="w", name=f"w{h}_{t}",
                            bufs=11)
                nc.scalar.activation(w[:], ps[:, 0:BAND],
                                     mybir.ActivationFunctionType.Exp)
                nc.vector.scalar_tensor_tensor(
                    w[:], w[:], 1.0, mask_s[:, t, :],
                    op0=mybir.AluOpType.mult, op1=mybir.AluOpType.mult,
                    accum_out=lsum4[:, t:t + 1])
                # per-tile recip + in-place normalize so the dependency
                # chain drains per-t instead of after all four tiles
                nc.vector.reciprocal(r4[:, t:t + 1], lsum4[:, t:t + 1])
                nc.vector.tensor_scalar_mul(w[:], w[:], r4[:, t:t + 1])
                wt_new.append(w)

        if h is not None and h + 1 < H:
            qproj(h + 1)

        if do_b and h is not None:
            po = ps_ot.tile([128, TLOC], f32, tag="ot", name=f"po{hp}")
            for t in range(NT):
                for b in range(5):
                    mm(po[:, t * 128:(t + 1) * 128],
                       vv[:, t + b, gb * 128:(gb + 1) * 128],
                       wT_list[t][:, b * 128:(b + 1) * 128],
                       start=(b == 0), stop=(b == 4))
            nc.scalar.copy(oT[:, hp, :], po[:])

        if h is not None:
            cur = wt_new
        iter_body.prev = cur

    iter_body.prev = None
    qproj(0)
    for h in range(H):
        iter_body(h)
    iter_body(None)  # drain: stage_b for h=15

    sm.release()
    wp.release()
    xtp.release()

    # ---------------- P3: output projection ----------------
    wop = tc.alloc_tile_pool(name="wop", bufs=4, side="right")
    ych_p = tc.alloc_tile_pool(name="ych_p", bufs=4, side="right")
    for dblk in range(4):
        woc = wop.tile([128, H, 512], bf16, tag="wo")
        nc.sync.dma_start(
            woc[:],
            wo_d.ap()[:, dblk * 512:(dblk + 1) * 512]
            .rearrange("(h p) e -> p h e", p=128))
        for t in range(NT):
            py = ps_ot.tile([128, 512], f32, tag="ot",
                            name=f"py{dblk}_{t}")
            for h in range(H):
                mm(py[:], oT[:, h, t * 128:(t + 1) * 128], woc[:, h, :],
                   start=(h == 0), stop=(h == H - 1))
            ych = ych_p.tile([128, 512], bf16, tag="ych")
            if t % 2 == 0:
                nc.vector.tensor_copy(ych[:], py[:])
            else:
                nc.scalar.copy(ych[:], py[:])
            nc.sync.dma_start(
                y_d.ap()[t * 128:(t + 1) * 128,
                         dblk * 512:(dblk + 1) * 512],
                ych[:])

    ych_p.release()
    wop.release()
    proj.release()

    if lp is not None:
        lp.__exit__(None, None, None)
        dtile = pers.tile([128, 128], f32, tag="dtile")
        nc.vector.memset(dtile[:], 0.0)
        nc.sync.dma_start(dummy_d.ap(), dtile[0:1, :])

    ps_ot.release()
    ps_pt.release()
    ps_s.release()
    pers.release()


def build_nc(loop_n=None):
    key = ("nc", loop_n)
    if key in _CACHE:
        return _CACHE[key]
    import concourse.bacc as bacc
    import concourse.mybir as mybir
    import concourse.tile as tile
    from concourse.masks import make_identity

    nc = bacc.Bacc("TRN2", target_bir_lowering=False, debug=False,
                   num_devices=N_CORES)
    with tile.TileContext(nc) as tc:
        _emit(nc, tc, tile, mybir, make_identity, loop_n=loop_n)
    nc.compile()
    _CACHE[key] = nc
    return nc


def _bf16(a):
    import ml_dtypes
    return np.asarray(a, np.float32).astype(ml_dtypes.bfloat16)


def make_inputs_for_core(c, xf, Wq, Wk, Wv, Wo):
    """xf: [T, D] float32 (already squeezed)."""
    if c == 0:
        x_c = np.concatenate(
            [np.zeros((WINDOW, D), np.float32), xf[:TLOC]], axis=0)
    else:
        x_c = xf[TLOC * c - WINDOW: TLOC * c + TLOC]
    # host-side transpose: xt[(dc p), j] = x_c[j, dc*128+p]
    xt = np.ascontiguousarray(x_c.T)

    jj = np.arange(BAND)[None, None, :]
    p = np.arange(128)[None, :, None]
    t = np.arange(NT)[:, None, None]
    allowed = (jj >= p) & (jj <= p + WINDOW)
    if c == 0:
        allowed = allowed & (128 * t + jj >= WINDOW)
    allowed = np.broadcast_to(allowed, (NT, 128, BAND))
    mask = np.where(allowed, np.float32(1.0), np.float32(0.0))

    return {
        "xt": _bf16(xt),
        "wq": _bf16(np.asarray(Wq, np.float32) * np.float32(SCALE)),
        "wk": _bf16(Wk),
        "wv": _bf16(Wv),
        "wo": _bf16(Wo),
        "mask": _bf16(mask),
    }


def kernel(x, Wq, Wk, Wv, Wo):
    from concourse.bass_utils import run_bass_kernel_spmd

    nc = build_nc()
    xf = np.asarray(x, np.float32).reshape(T, D)
    in_maps = [make_inputs_for_core(c, xf, Wq, Wk, Wv, Wo)
               for c in range(N_CORES)]
    res = run_bass_kernel_spmd(nc, in_maps, core_ids=list(range(N_CORES)))
    y = np.concatenate(
        [res.results[c]["y"].astype(np.float32) for c in range(N_CORES)],
        axis=0)
    return y.reshape(1, T, D)


# revision 4
# speedup vs baseline: 1.0192x; 1.0192x over previous
"""Trainium2 Bass kernel: GQA sliding-window attention (bf16 redesign).

Problem: B=1, T=4096, D=2048, H=16 q-heads, KVH=4 kv-heads, HD=128,
causal sliding window 512.

Sharding: 8-way sequence parallel. Core c owns query rows
[512c, 512c+512). It receives xT columns for rows [512(c-1), 512(c+1))
(halo of 512 rows; core 0's halo is zeros and is masked out). Weights
replicated. Outputs are disjoint row blocks -> concatenation.

All compute in bf16 (f32 PSUM accumulation). Host pre-transposes x
(so no on-device transpose phase), pre-scales Wq by SCALE, and casts
everything to bf16.

Per-core layouts (SBUF partition dim first):
  xT  [128, 16, 1024] bf16 : xT[p, dc, j] = x[j, 128*dc+p]
  kT  [128, 4, 1024]  bf16 : kT[p, g, j]  = k[j, 128*g+p]
  vv  [128, 8, 512]   bf16 : vv[p, jc, e] = v[128*jc+p, e]
  qT  [128, 16, 512]  bf16 : qT[p, h, i]  = q[i, 128*h+p] (SCALE folded
    into Wq host-side)
  Scores per (h, t): s[i', jj], key j = 128*t + jj, jj in [0,640).
  Softmax without max-subtraction (|scores| < ~6 for this input
    distribution, verified host-side in the test harness).
  Multiplicative {0,1} bf16 mask post-exp, fused with the row-sum
    (scalar_tensor_tensor accum_out).
  Normalization: w *= 1/rowsum on DVE (bf16 4x mode) before the PE
    transpose. (The PE transpose rhs must be a permutation matrix, so
    normalization cannot fold into it.)
  PV per (h, t): 5 chunk matmuls over the 640-key band into a
    [128, 128] window of the per-head po accumulator.
  oT  [128, 16, 512] bf16 ; y = oT.T @ Wo streamed in 512-col blocks.

P1 is paced against the xT DMA chunks: k-projections for g0/g1/g2 keep
6 accumulation groups open (4 ps_s half-windows + 2 ps_ot) and consume
each 4-dc xT chunk as it lands; g3 + v run afterwards from SBUF.
"""

import numpy as np

T = 4096
D = 2048
H = 16
KVH = 4
HD = 128
WINDOW = 512
SCALE = HD ** -0.5
N_CORES = 8
TLOC = T // N_CORES          # 512 own query rows / core
XROWS = TLOC + WINDOW        # 1024 x rows / core (halo + own)
NT = TLOC // 128             # 4 q-tiles of 128 rows
NJC = XROWS // 128           # 8 key chunks of 128
BAND = WINDOW + 128          # 640 key columns per q-tile
DC = D // 128                # 16 d-chunks

_CACHE = {}


def _emit(nc, tc, tile, mybir, make_identity, loop_n=None):
    f32 = mybir.dt.float32
    bf16 = mybir.dt.bfloat16

    timing = loop_n is not None
    kin = "Internal" if timing else "ExternalInput"
    kout = "Internal" if timing else "ExternalOutput"
    # xt: x transposed host-side, [(dc p), j] = x[j, 128*dc+p]
    xt_d = nc.dram_tensor("xt", [D, XROWS], bf16, kind=kin)
    wq_d = nc.dram_tensor("wq", [D, H * HD], bf16, kind=kin)
    wk_d = nc.dram_tensor("wk", [D, KVH * HD], bf16, kind=kin)
    wv_d = nc.dram_tensor("wv", [D, KVH * HD], bf16, kind=kin)
    wo_d = nc.dram_tensor("wo", [H * HD, D], bf16, kind=kin)
    mask_d = nc.dram_tensor("mask", [NT, 128, BAND], bf16, kind=kin)
    y_d = nc.dram_tensor("y", [TLOC, D], bf16, kind=kout)
    if timing:
        dummy_d = nc.dram_tensor("bench_done", [1, 128], f32,
                                 kind="ExternalOutput")

    def mm(out, lhsT, rhs, start, stop):
        nc.tensor.matmul(out, lhsT, rhs, start=start, stop=stop)

    # --- persistent pools (outside timing loop) ---
    # PSUM budget (8 banks): ps_s = 2 bufs x [128,1024]f32 (2 banks each)
    # = 4 banks; ps_pt = 2 bufs x [128,640]bf16 (1 bank each) = 2 banks;
    # ps_ot = 2 bufs x [128,512]f32 = 2 banks.
    pers = tc.alloc_tile_pool(name="pers", bufs=1)
    ps_s = tc.alloc_tile_pool(name="ps_s", bufs=2, space="PSUM")
    ps_pt = tc.alloc_tile_pool(name="ps_pt", bufs=2, space="PSUM")
    ps_ot = tc.alloc_tile_pool(name="ps_ot", bufs=2, space="PSUM")

    identb = pers.tile([128, 128], bf16, tag="identb")
    make_identity(nc, identb[:])

    lp = tc.For_i(0, loop_n, 1) if timing else None
    if lp is not None:
        lp.__enter__()

    proj = tc.alloc_tile_pool(name="proj", bufs=1)
    mask_s = proj.tile([128, NT, BAND], bf16, tag="mask")
    qT = proj.tile([128, H, TLOC], bf16, tag="qT")
    kT = proj.tile([128, KVH, XROWS], bf16, tag="kT")
    vv = proj.tile([128, NJC, KVH * HD], bf16, tag="vv")
    oT = proj.tile([128, H, TLOC], bf16, tag="oT")
    xtp = tc.alloc_tile_pool(name="xtp", bufs=1)
    xT = xtp.tile([128, DC, XROWS], bf16, tag="xT")

    # weight pools (right side; kvw above wp/wop so kvw frees after P1).
    # 2 rotating bufs each: the in-order sync DMA queue stalls on the
    # 3rd wq chunk until qproj(3) frees its buffer, which still lands
    # far ahead of its consumer.
    wp = tc.alloc_tile_pool(name="wp", bufs=2, side="right")
    wop = tc.alloc_tile_pool(name="wop", bufs=2, side="right")
    kvw = tc.alloc_tile_pool(name="kvw", bufs=1, side="right")

    # ---------------- DMA issue (sync queue, in need-order) -------------
    # Order paces P1: wk(g0,g1) -> xt chunks interleaved with wk(g2,g3).
    wk_s = kvw.tile([128, DC, KVH * HD], bf16, tag="wk")
    wv_s = kvw.tile([128, DC, KVH * HD], bf16, tag="wv")
    # Small first pieces so PE starts ~5us in: wk rows for the first 4
    # dc-chunks only (g0/g1 cols), then the first 2-dc xT chunk; the
    # rest of wk lands while those are consumed.
    nc.sync.dma_start(
        wk_s[:, 0:4, 0:256],
        wk_d.ap()[0:512, 0:256].rearrange("(c p) e -> p c e", p=128))
    for lo, hi in ((0, 2), (2, 4), (4, 8), (8, 12), (12, 16)):
        nc.sync.dma_start(
            xT[:, lo:hi, :],
            xt_d.ap()[lo * 128:hi * 128, :]
            .rearrange("(c p) j -> p c j", p=128))
        if hi == 2:
            nc.sync.dma_start(
                wk_s[:, 0:4, 256:512],
                wk_d.ap()[0:512, 256:512]
                .rearrange("(c p) e -> p c e", p=128))
        elif hi == 4:
            nc.sync.dma_start(
                wk_s[:, 4:8, :],
                wk_d.ap()[512:1024, :].rearrange("(c p) e -> p c e", p=128))
        elif hi == 8:
            nc.sync.dma_start(
                wk_s[:, 8:16, :],
                wk_d.ap()[1024:2048, :]
                .rearrange("(c p) e -> p c e", p=128))
    nc.sync.dma_start(
        wv_s[:], wv_d.ap().rearrange("(c p) e -> p c e", p=128))
    wq_c = []
    for cq in range(4):
        wqc = wp.tile([128, DC, 512], bf16, tag="wq", name=f"wq{cq}")
        nc.sync.dma_start(
            wqc[:],
            wq_d.ap()[:, cq * 512:(cq + 1) * 512]
            .rearrange("(c p) e -> p c e", p=128))
        wq_c.append(wqc)
        if cq == 0:
            nc.sync.dma_start(mask_s[:],
                              mask_d.ap().rearrange("t p j -> p t j"))
    # wo block 0 prefetched here: its dblk-0 accumulation groups start
    # right after the last scores iteration (see early-P3 fill below)
    woc0 = wop.tile([128, H, 512], bf16, tag="wo", name="wo0")
    nc.sync.dma_start(
        woc0[:], wo_d.ap()[:, 0:512].rearrange("(h p) e -> p h e", p=128))

    # ---------------- P1: k/v projections ----------------
    # P1a: k for g0..g2, 6 open accumulation groups, paced by xT chunks.
    sA = ps_s.tile([128, 1024], f32, tag="score", name="p1_sA")
    sB = ps_s.tile([128, 1024], f32, tag="score", name="p1_sB")
    oA = ps_ot.tile([128, 512], f32, tag="ot", name="p1_oA")
    oB = ps_ot.tile([128, 512], f32, tag="ot", name="p1_oB")
    kacc = {  # (g, half) -> psum window
        (0, 0): sA[:, 0:512], (0, 1): sA[:, 512:1024],
        (1, 0): sB[:, 0:512], (1, 1): sB[:, 512:1024],
        (2, 0): oA[:], (2, 1): oB[:],
    }
    stages = [((0, 2), (0, 1, 2)), ((2, 4), (0, 1, 2)),
              ((4, 8), (0, 1, 2)), ((8, 12), (0, 1, 2)),
              ((12, 16), (0, 1, 2))]
    for (lo, hi), gs in stages:
        for g in gs:
            for half in range(2):
                for dc in range(lo, hi):
                    mm(kacc[(g, half)],
                       wk_s[:, dc, g * 128:(g + 1) * 128],
                       xT[:, dc, half * 512:(half + 1) * 512],
                       start=(dc == 0), stop=(dc == DC - 1))
    for i, ((g, half), acc) in enumerate(kacc.items()):
        if i % 2 == 0:
            nc.vector.tensor_copy(
                kT[:, g, half * 512:(half + 1) * 512], acc)
        else:
            nc.scalar.copy(kT[:, g, half * 512:(half + 1) * 512], acc)

    # P1b: k g3 + v, xT fully resident.
    for half in range(2):
        pk = ps_ot.tile([128, 512], f32, tag="ot", name=f"pk3_{half}")
        for dc in range(DC):
            mm(pk[:], wk_s[:, dc, 384:512],
               xT[:, dc, half * 512:(half + 1) * 512],
               start=(dc == 0), stop=(dc == DC - 1))
        if half == 0:
            nc.vector.tensor_copy(kT[:, 3, 0:512], pk[:])
        else:
            nc.scalar.copy(kT[:, 3, 512:1024], pk[:])

    for jc in range(NJC):
        pv = ps_ot.tile([128, 512], f32, tag="ot", name=f"pv{jc}")
        for dc in range(DC):
            mm(pv[:], xT[:, dc, jc * 128:(jc + 1) * 128], wv_s[:, dc, :],
               start=(dc == 0), stop=(dc == DC - 1))
        if jc % 2 == 0:
            nc.vector.tensor_copy(vv[:, jc, :], pv[:])
        else:
            nc.scalar.copy(vv[:, jc, :], pv[:])

    kvw.release()

    # ---------------- P2: attention, q projection interleaved ----------
    sm = tc.alloc_tile_pool(name="sm", bufs=2, side="right")

    def qproj(h, dve_copy=False):
        """q projection for head h (runs one head ahead of scores)."""
        wqc = wq_c[h // 4]
        e0 = (h % 4) * 128
        pq = ps_ot.tile([128, TLOC], f32, tag="ot", name=f"pq{h}")
        for dc in range(DC):
            mm(pq[:], wqc[:, dc, e0:e0 + 128], xT[:, dc, WINDOW:XROWS],
               start=(dc == 0), stop=(dc == DC - 1))
        if dve_copy:
            nc.vector.tensor_copy(qT[:, h, :], pq[:])
        else:
            nc.scalar.copy(qT[:, h, :], pq[:])

    def stage_a(h):
        """scores + softmax for head h (qT already resident), with the
        mask+rowsum on the otherwise-idle Pool engine. Used only for the
        pulled-forward last head, whose results are not needed for a
        full iteration (so the slower gpsimd stt is off anyone's
        critical path and DVE stays free for the main head's softmax)."""
        g = h // (H // KVH)
        lsum4 = sm.tile([128, NT], f32, tag="l4", name=f"l4_{h}", bufs=2)
        r4 = sm.tile([128, NT], f32, tag="r4", name=f"r4_{h}", bufs=2)
        wt_new = []
        for t in range(NT):
            ps = ps_s.tile([128, 1024], f32, tag="score", name=f"s{h}_{t}")
            mm(ps[:, 0:512], qT[:, h, t * 128:(t + 1) * 128],
               kT[:, g, t * 128:t * 128 + 512], start=True, stop=True)
            mm(ps[:, 512:BAND], qT[:, h, t * 128:(t + 1) * 128],
               kT[:, g, t * 128 + 512:t * 128 + BAND], start=True,
               stop=True)
            w = sm.tile([128, BAND], bf16, tag="w", name=f"w{h}_{t}",
                        bufs=14)
            nc.scalar.activation(w[:], ps[:, 0:BAND],
                                 mybir.ActivationFunctionType.Exp)
            nc.gpsimd.scalar_tensor_tensor(
                w[:], w[:], 1.0, mask_s[:, t, :],
                op0=mybir.AluOpType.mult, op1=mybir.AluOpType.mult,
                accum_out=lsum4[:, t:t + 1])
            nc.vector.reciprocal(r4[:, t:t + 1], lsum4[:, t:t + 1])
            nc.vector.tensor_scalar_mul(w[:], w[:], r4[:, t:t + 1])
            wt_new.append(w)
        return wt_new

    def iter_body(h, hp_override=None, extra=None):
        """Emit one pipeline iteration.

        h: head whose scores/softmax run this iteration (None past end).
        hp_override: stage_b head for drain iterations.
        extra: additional scores head emitted after this iteration's PV.
        """
        if h is not None:
            hp = h - 1
        elif hp_override is not None:
            hp = hp_override
        else:
            hp = H - 1
        do_b = hp >= 0 and iter_body.prev is not None
        if do_b:
            wt_list = iter_body.prev
            gb = hp // (H // KVH)
            wT_list = []
        cur = None
        if h is not None:
            g = h // (H // KVH)
            lsum4 = sm.tile([128, NT], f32, tag="l4", name=f"l4_{h}",
                            bufs=2)
            r4 = sm.tile([128, NT], f32, tag="r4", name=f"r4_{h}", bufs=2)
            wt_new = []

        if extra is not None:
            # q-projection of the extra head up front: its PE work leads
            # the iteration and the DVE copy lands before ACT finishes
            # this head's exps, so stage_a(extra) below never stalls
            qproj(extra, dve_copy=True)

        # interleave: transposes(hp, t) with scores(h, t). On the last
        # scores head, emit scores before transposes so the softmax
        # chain of h=15 starts early (shortens the pipeline drain).
        order = ("b", "a") if h != H - 1 else ("a", "b")
        for t in range(NT):
          for phase in order:
            if phase == "b" and do_b:
                pt = ps_pt.tile([128, BAND], bf16, tag="pt",
                                name=f"pt{hp}_{t}")
                for b in range(5):
                    nc.tensor.transpose(
                        pt[:, b * 128:(b + 1) * 128],
                        wt_list[t][:, b * 128:(b + 1) * 128],
                        identb[:])
                wT = sm.tile([128, BAND], bf16, tag="wT",
                             name=f"wT{hp}_{t}", bufs=3)
                nc.vector.tensor_copy(wT[:], pt[:])
                wT_list.append(wT)
                if h is None:
                    # drain iteration: no scores/qproj work to hide the
                    # copy latency, so run each PV right after its tile
                    if t == 0:
                        po_d = ps_ot.tile([128, TLOC], f32, tag="ot",
                                          name=f"po{hp}")
                    for b in range(5):
                        mm(po_d[:, t * 128:(t + 1) * 128],
                           vv[:, t + b, gb * 128:(gb + 1) * 128],
                           wT[:, b * 128:(b + 1) * 128],
                           start=(b == 0), stop=(b == 4))
                    if t == NT - 1:
                        nc.scalar.copy(oT[:, hp, :], po_d[:])
            if phase == "a" and h is not None:
                ps = ps_s.tile([128, 1024], f32, tag="score",
                               name=f"s{h}_{t}")
                mm(ps[:, 0:512], qT[:, h, t * 128:(t + 1) * 128],
                   kT[:, g, t * 128:t * 128 + 512], start=True, stop=True)
                mm(ps[:, 512:BAND], qT[:, h, t * 128:(t + 1) * 128],
                   kT[:, g, t * 128 + 512:t * 128 + BAND], start=True,
                   stop=True)
                w = sm.tile([128, BAND], bf16, tag="w", name=f"w{h}_{t}",
                            bufs=14)
                nc.scalar.activation(w[:], ps[:, 0:BAND],
                                     mybir.ActivationFunctionType.Exp)
                nc.vector.scalar_tensor_tensor(
                    w[:], w[:], 1.0, mask_s[:, t, :],
                    op0=mybir.AluOpType.mult, op1=mybir.AluOpType.mult,
                    accum_out=lsum4[:, t:t + 1])
                # per-tile recip + in-place normalize so the dependency
                # chain drains per-t instead of after all four tiles
                nc.vector.reciprocal(r4[:, t:t + 1], lsum4[:, t:t + 1])
                nc.vector.tensor_scalar_mul(w[:], w[:], r4[:, t:t + 1])
                wt_new.append(w)

        if h is not None and h + 1 < H and h + 1 != extra:
            qproj(h + 1)

        if do_b and h is not None:
            po = ps_ot.tile([128, TLOC], f32, tag="ot", name=f"po{hp}")
            for t in range(NT):
                for b in range(5):
                    mm(po[:, t * 128:(t + 1) * 128],
                       vv[:, t + b, gb * 128:(gb + 1) * 128],
                       wT_list[t][:, b * 128:(b + 1) * 128],
                       start=(b == 0), stop=(b == 4))
            nc.scalar.copy(oT[:, hp, :], po[:])

        if extra is not None:
            # softmax of the extra head runs here, a full iteration
            # before its stage_b, so the final b-only iterations find
            # everything ready (no pipeline-drain stalls)
            iter_body.extra_prev = stage_a(extra)

        if h is not None:
            cur = wt_new
        iter_body.prev = cur

    iter_body.prev = None
    iter_body.extra_prev = None
    qproj(0)
    if True:  # simple pipeline: scores h / stage_b h-1, single drain
        for h in range(H):
            iter_body(h)
        # early-P3 fill: open the dblk-0 output-projection groups for
        # heads 0..14 in the now-idle ps_s banks. This PE work fills the
        # pipeline-drain stalls (the final head's softmax chain and wT
        # copies); head 15 + stop land in P3 proper. Same accumulation
        # order as before -> bit-identical results.
        py_a = ps_s.tile([128, 1024], f32, tag="score", name="py_a")
        py_b = ps_s.tile([128, 1024], f32, tag="score", name="py_b")
        py0 = [py_a[:, 0:512], py_a[:, 512:1024],
               py_b[:, 0:512], py_b[:, 512:1024]]
        for t in range(NT):
            for h in range(H - 1):
                mm(py0[t], oT[:, h, t * 128:(t + 1) * 128],
                   woc0[:, h, :], start=(h == 0), stop=False)
        iter_body(None, hp_override=H - 1)
    else:  # pulled-forward variant (kept for reference; slightly slower)
        for h in range(H - 1):
            iter_body(h, extra=(H - 1 if h == H - 2 else None))
        iter_body(None, hp_override=H - 2)
        iter_body.prev = iter_body.extra_prev
        iter_body(None, hp_override=H - 1)

    sm.release()
    xtp.release()

    # ---------------- P3: output projection ----------------
    ych_p = tc.alloc_tile_pool(name="ych_p", bufs=4, side="right")
    for dblk in range(4):
        if dblk == 0:
            woc = woc0
        else:
            woc = wop.tile([128, H, 512], bf16, tag="wo")
            nc.sync.dma_start(
                woc[:],
                wo_d.ap()[:, dblk * 512:(dblk + 1) * 512]
                .rearrange("(h p) e -> p h e", p=128))
        for t in range(NT):
            if dblk == 0:
                py = py0[t]
                mm(py, oT[:, H - 1, t * 128:(t + 1) * 128],
                   woc[:, H - 1, :], start=False, stop=True)
            else:
                py = ps_ot.tile([128, 512], f32, tag="ot",
                                name=f"py{dblk}_{t}")
                for h in range(H):
                    mm(py[:], oT[:, h, t * 128:(t + 1) * 128],
                       woc[:, h, :], start=(h == 0), stop=(h == H - 1))
            ych = ych_p.tile([128, 512], bf16, tag="ych")
            if t % 2 == 0:
                nc.vector.tensor_copy(ych[:], py if dblk == 0 else py[:])
            else:
                nc.scalar.copy(ych[:], py if dblk == 0 else py[:])
            nc.sync.dma_start(
                y_d.ap()[t * 128:(t + 1) * 128,
                         dblk * 512:(dblk + 1) * 512],
                ych[:])

    ych_p.release()
    wop.release()
    wp.release()
    proj.release()

    if lp is not None:
        lp.__exit__(None, None, None)
        dtile = pers.tile([128, 128], f32, tag="dtile")
        nc.vector.memset(dtile[:], 0.0)
        nc.sync.dma_start(dummy_d.ap(), dtile[0:1, :])

    ps_ot.release()
    ps_pt.release()
    ps_s.release()
    pers.release()


def build_nc(loop_n=None):
    key = ("nc", loop_n)
    if key in _CACHE:
        return _CACHE[key]
    import concourse.bacc as bacc
    import concourse.mybir as mybir
    import concourse.tile as tile
    from concourse.masks import make_identity

    nc = bacc.Bacc("TRN2", target_bir_lowering=False, debug=False,
                   num_devices=N_CORES)
    with tile.TileContext(nc) as tc:
        _emit(nc, tc, tile, mybir, make_identity, loop_n=loop_n)
    nc.compile()
    _CACHE[key] = nc
    return nc


def _bf16(a):
    import ml_dtypes
    return np.asarray(a, np.float32).astype(ml_dtypes.bfloat16)


def make_inputs_for_core(c, xf, Wq, Wk, Wv, Wo):
    """xf: [T, D] float32 (already squeezed)."""
    if c == 0:
        x_c = np.concatenate(
            [np.zeros((WINDOW, D), np.float32), xf[:TLOC]], axis=0)
    else:
        x_c = xf[TLOC * c - WINDOW: TLOC * c + TLOC]
    # host-side transpose: xt[(dc p), j] = x_c[j, dc*128+p]
    xt = np.ascontiguousarray(x_c.T)

    jj = np.arange(BAND)[None, None, :]
    p = np.arange(128)[None, :, None]
    t = np.arange(NT)[:, None, None]
    allowed = (jj >= p) & (jj <= p + WINDOW)
    if c == 0:
        allowed = allowed & (128 * t + jj >= WINDOW)
    allowed = np.broadcast_to(allowed, (NT, 128, BAND))
    mask = np.where(allowed, np.float32(1.0), np.float32(0.0))

    return {
        "xt": _bf16(xt),
        "wq": _bf16(np.asarray(Wq, np.float32) * np.float32(SCALE)),
        "wk": _bf16(Wk),
        "wv": _bf16(Wv),
        "wo": _bf16(Wo),
        "mask": _bf16(mask),
    }


def kernel(x, Wq, Wk, Wv, Wo):
    from concourse.bass_utils import run_bass_kernel_spmd

    nc = build_nc()
    xf = np.asarray(x, np.float32).reshape(T, D)
    in_maps = [make_inputs_for_core(c, xf, Wq, Wk, Wv, Wo)
               for c in range(N_CORES)]
    res = run_bass_kernel_spmd(nc, in_maps, core_ids=list(range(N_CORES)))
    y = np.concatenate(
        [res.results[c]["y"].astype(np.float32) for c in range(N_CORES)],
        axis=0)
    return y.reshape(1, T, D)


# revision 5
# speedup vs baseline: 1.0265x; 1.0072x over previous
"""Trainium2 Bass kernel: GQA sliding-window attention (bf16 redesign).

Problem: B=1, T=4096, D=2048, H=16 q-heads, KVH=4 kv-heads, HD=128,
causal sliding window 512.

Sharding: 8-way sequence parallel. Core c owns query rows
[512c, 512c+512). It receives xT columns for rows [512(c-1), 512(c+1))
(halo of 512 rows; core 0's halo is zeros and is masked out). Weights
replicated. Outputs are disjoint row blocks -> concatenation.

All compute in bf16 (f32 PSUM accumulation). Host pre-transposes x
(so no on-device transpose phase), pre-scales Wq by SCALE, and casts
everything to bf16.

Per-core layouts (SBUF partition dim first):
  xT  [128, 16, 1024] bf16 : xT[p, dc, j] = x[j, 128*dc+p]
  kT  [128, 4, 1024]  bf16 : kT[p, g, j]  = k[j, 128*g+p]
  vv  [128, 8, 512]   bf16 : vv[p, jc, e] = v[128*jc+p, e]
  qT  [128, 16, 512]  bf16 : qT[p, h, i]  = q[i, 128*h+p] (SCALE folded
    into Wq host-side)
  Scores per (h, t): s[i', jj], key j = 128*t + jj, jj in [0,640).
  Softmax without max-subtraction (|scores| < ~6 for this input
    distribution, verified host-side in the test harness).
  Multiplicative {0,1} bf16 mask post-exp, fused with the row-sum
    (scalar_tensor_tensor accum_out).
  Normalization: w *= 1/rowsum on DVE (bf16 4x mode) before the PE
    transpose. (The PE transpose rhs must be a permutation matrix, so
    normalization cannot fold into it.)
  PV per (h, t): 5 chunk matmuls over the 640-key band into a
    [128, 128] window of the per-head po accumulator.
  oT  [128, 16, 512] bf16 ; y = oT.T @ Wo streamed in 512-col blocks.

P1 is paced against the xT DMA chunks: k-projections for g0/g1/g2 keep
6 accumulation groups open (4 ps_s half-windows + 2 ps_ot) and consume
each 4-dc xT chunk as it lands; g3 + v run afterwards from SBUF.
"""

import numpy as np

T = 4096
D = 2048
H = 16
KVH = 4
HD = 128
WINDOW = 512
SCALE = HD ** -0.5
N_CORES = 8
TLOC = T // N_CORES          # 512 own query rows / core
XROWS = TLOC + WINDOW        # 1024 x rows / core (halo + own)
NT = TLOC // 128             # 4 q-tiles of 128 rows
NJC = XROWS // 128           # 8 key chunks of 128
BAND = WINDOW + 128          # 640 key columns per q-tile
DC = D // 128                # 16 d-chunks

_CACHE = {}


def _emit(nc, tc, tile, mybir, make_identity, loop_n=None):
    f32 = mybir.dt.float32
    bf16 = mybir.dt.bfloat16

    timing = loop_n is not None
    kin = "Internal" if timing else "ExternalInput"
    kout = "Internal" if timing else "ExternalOutput"
    # xt: x transposed host-side, [(dc p), j] = x[j, 128*dc+p]
    xt_d = nc.dram_tensor("xt", [D, XROWS], bf16, kind=kin)
    wq_d = nc.dram_tensor("wq", [D, H * HD], bf16, kind=kin)
    wk_d = nc.dram_tensor("wk", [D, KVH * HD], bf16, kind=kin)
    wv_d = nc.dram_tensor("wv", [D, KVH * HD], bf16, kind=kin)
    wo_d = nc.dram_tensor("wo", [H * HD, D], bf16, kind=kin)
    mask_d = nc.dram_tensor("mask", [NT, 128, BAND], bf16, kind=kin)
    y_d = nc.dram_tensor("y", [TLOC, D], bf16, kind=kout)
    if timing:
        dummy_d = nc.dram_tensor("bench_done", [1, 128], f32,
                                 kind="ExternalOutput")

    def mm(out, lhsT, rhs, start, stop):
        nc.tensor.matmul(out, lhsT, rhs, start=start, stop=stop)

    # --- persistent pools (outside timing loop) ---
    # PSUM budget (8 banks): ps_s = 2 bufs x [128,1024]f32 (2 banks each)
    # = 4 banks; ps_pt = 2 bufs x [128,640]bf16 (1 bank each) = 2 banks;
    # ps_ot = 2 bufs x [128,512]f32 = 2 banks.
    pers = tc.alloc_tile_pool(name="pers", bufs=1)
    ps_s = tc.alloc_tile_pool(name="ps_s", bufs=2, space="PSUM")
    ps_pt = tc.alloc_tile_pool(name="ps_pt", bufs=2, space="PSUM")
    ps_ot = tc.alloc_tile_pool(name="ps_ot", bufs=2, space="PSUM")

    identb = pers.tile([128, 128], bf16, tag="identb")
    make_identity(nc, identb[:])

    lp = tc.For_i(0, loop_n, 1) if timing else None
    if lp is not None:
        lp.__enter__()

    proj = tc.alloc_tile_pool(name="proj", bufs=1)
    mask_s = proj.tile([128, NT, BAND], bf16, tag="mask")
    qT = proj.tile([128, H, TLOC], bf16, tag="qT")
    kT = proj.tile([128, KVH, XROWS], bf16, tag="kT")
    vv = proj.tile([128, NJC, KVH * HD], bf16, tag="vv")
    oT = proj.tile([128, H, TLOC], bf16, tag="oT")
    xtp = tc.alloc_tile_pool(name="xtp", bufs=1)
    xT = xtp.tile([128, DC, XROWS], bf16, tag="xT")

    # weight pools (right side; kvw above wp/wop so kvw frees after P1).
    # 2 rotating bufs each: the in-order sync DMA queue stalls on the
    # 3rd wq chunk until qproj(3) frees its buffer, which still lands
    # far ahead of its consumer.
    wp = tc.alloc_tile_pool(name="wp", bufs=2, side="right")
    wop = tc.alloc_tile_pool(name="wop", bufs=2, side="right")
    kvw = tc.alloc_tile_pool(name="kvw", bufs=1, side="right")

    # ---------------- DMA issue (sync queue, in need-order) -------------
    # Order paces P1: wk(g0,g1) -> xt chunks interleaved with wk(g2,g3).
    wk_s = kvw.tile([128, DC, KVH * HD], bf16, tag="wk")
    wv_s = kvw.tile([128, DC, KVH * HD], bf16, tag="wv")
    # Small first pieces so PE starts ~5us in: wk rows for the first 4
    # dc-chunks only (g0/g1 cols), then the first 2-dc xT chunk; the
    # rest of wk lands while those are consumed.
    nc.sync.dma_start(
        wk_s[:, 0:4, 0:256],
        wk_d.ap()[0:512, 0:256].rearrange("(c p) e -> p c e", p=128))
    for lo, hi in ((0, 2), (2, 4), (4, 8), (8, 12), (12, 16)):
        nc.sync.dma_start(
            xT[:, lo:hi, :],
            xt_d.ap()[lo * 128:hi * 128, :]
            .rearrange("(c p) j -> p c j", p=128))
        if hi == 2:
            nc.sync.dma_start(
                wk_s[:, 0:4, 256:512],
                wk_d.ap()[0:512, 256:512]
                .rearrange("(c p) e -> p c e", p=128))
        elif hi == 4:
            nc.sync.dma_start(
                wk_s[:, 4:8, :],
                wk_d.ap()[512:1024, :].rearrange("(c p) e -> p c e", p=128))
        elif hi == 8:
            nc.sync.dma_start(
                wk_s[:, 8:16, :],
                wk_d.ap()[1024:2048, :]
                .rearrange("(c p) e -> p c e", p=128))
    nc.sync.dma_start(
        wv_s[:], wv_d.ap().rearrange("(c p) e -> p c e", p=128))
    wq_c = []
    for cq in range(4):
        wqc = wp.tile([128, DC, 512], bf16, tag="wq", name=f"wq{cq}")
        nc.sync.dma_start(
            wqc[:],
            wq_d.ap()[:, cq * 512:(cq + 1) * 512]
            .rearrange("(c p) e -> p c e", p=128))
        wq_c.append(wqc)
        if cq == 0:
            nc.sync.dma_start(mask_s[:],
                              mask_d.ap().rearrange("t p j -> p t j"))
    # wo block 0 prefetched here: its dblk-0 accumulation groups start
    # right after the last scores iteration (see early-P3 fill below)
    woc0 = wop.tile([128, H, 512], bf16, tag="wo", name="wo0")
    nc.sync.dma_start(
        woc0[:], wo_d.ap()[:, 0:512].rearrange("(h p) e -> p h e", p=128))

    # ---------------- P1: k/v projections ----------------
    # P1a: k for g0..g2, 6 open accumulation groups, paced by xT chunks.
    sA = ps_s.tile([128, 1024], f32, tag="score", name="p1_sA")
    sB = ps_s.tile([128, 1024], f32, tag="score", name="p1_sB")
    oA = ps_ot.tile([128, 512], f32, tag="ot", name="p1_oA")
    oB = ps_ot.tile([128, 512], f32, tag="ot", name="p1_oB")
    kacc = {  # (g, half) -> psum window
        (0, 0): sA[:, 0:512], (0, 1): sA[:, 512:1024],
        (1, 0): sB[:, 0:512], (1, 1): sB[:, 512:1024],
        (2, 0): oA[:], (2, 1): oB[:],
    }
    stages = [((0, 2), (0, 1, 2)), ((2, 4), (0, 1, 2)),
              ((4, 8), (0, 1, 2)), ((8, 12), (0, 1, 2)),
              ((12, 16), (0, 1, 2))]
    for (lo, hi), gs in stages:
        for g in gs:
            for half in range(2):
                for dc in range(lo, hi):
                    mm(kacc[(g, half)],
                       wk_s[:, dc, g * 128:(g + 1) * 128],
                       xT[:, dc, half * 512:(half + 1) * 512],
                       start=(dc == 0), stop=(dc == DC - 1))
    for i, ((g, half), acc) in enumerate(kacc.items()):
        if i % 2 == 0:
            nc.vector.tensor_copy(
                kT[:, g, half * 512:(half + 1) * 512], acc)
        else:
            nc.scalar.copy(kT[:, g, half * 512:(half + 1) * 512], acc)

    # P1b: k g3 + v, xT fully resident.
    for half in range(2):
        pk = ps_ot.tile([128, 512], f32, tag="ot", name=f"pk3_{half}")
        for dc in range(DC):
            mm(pk[:], wk_s[:, dc, 384:512],
               xT[:, dc, half * 512:(half + 1) * 512],
               start=(dc == 0), stop=(dc == DC - 1))
        if half == 0:
            nc.vector.tensor_copy(kT[:, 3, 0:512], pk[:])
        else:
            nc.scalar.copy(kT[:, 3, 512:1024], pk[:])

    for jc in range(NJC):
        pv = ps_ot.tile([128, 512], f32, tag="ot", name=f"pv{jc}")
        for dc in range(DC):
            mm(pv[:], xT[:, dc, jc * 128:(jc + 1) * 128], wv_s[:, dc, :],
               start=(dc == 0), stop=(dc == DC - 1))
        if jc % 2 == 0:
            nc.vector.tensor_copy(vv[:, jc, :], pv[:])
        else:
            nc.scalar.copy(vv[:, jc, :], pv[:])

    kvw.release()

    # ---------------- P2: attention, q projection interleaved ----------
    sm = tc.alloc_tile_pool(name="sm", bufs=2, side="right")

    def qproj(h, dve_copy=False):
        """q projection for head h (runs one head ahead of scores)."""
        wqc = wq_c[h // 4]
        e0 = (h % 4) * 128
        pq = ps_ot.tile([128, TLOC], f32, tag="ot", name=f"pq{h}")
        for dc in range(DC):
            mm(pq[:], wqc[:, dc, e0:e0 + 128], xT[:, dc, WINDOW:XROWS],
               start=(dc == 0), stop=(dc == DC - 1))
        if dve_copy:
            nc.vector.tensor_copy(qT[:, h, :], pq[:])
        else:
            nc.scalar.copy(qT[:, h, :], pq[:])

    def stage_a(h):
        """scores + softmax for head h (qT already resident), with the
        mask+rowsum on the otherwise-idle Pool engine. Used only for the
        pulled-forward last head, whose results are not needed for a
        full iteration (so the slower gpsimd stt is off anyone's
        critical path and DVE stays free for the main head's softmax)."""
        g = h // (H // KVH)
        lsum4 = sm.tile([128, NT], f32, tag="l4", name=f"l4_{h}", bufs=2)
        r4 = sm.tile([128, NT], f32, tag="r4", name=f"r4_{h}", bufs=2)
        wt_new = []
        for t in range(NT):
            ps = ps_s.tile([128, 1024], f32, tag="score", name=f"s{h}_{t}")
            mm(ps[:, 0:512], qT[:, h, t * 128:(t + 1) * 128],
               kT[:, g, t * 128:t * 128 + 512], start=True, stop=True)
            mm(ps[:, 512:BAND], qT[:, h, t * 128:(t + 1) * 128],
               kT[:, g, t * 128 + 512:t * 128 + BAND], start=True,
               stop=True)
            w = sm.tile([128, BAND], bf16, tag="w", name=f"w{h}_{t}",
                        bufs=14)
            nc.scalar.activation(w[:], ps[:, 0:BAND],
                                 mybir.ActivationFunctionType.Exp)
            nc.gpsimd.scalar_tensor_tensor(
                w[:], w[:], 1.0, mask_s[:, t, :],
                op0=mybir.AluOpType.mult, op1=mybir.AluOpType.mult,
                accum_out=lsum4[:, t:t + 1])
            nc.vector.reciprocal(r4[:, t:t + 1], lsum4[:, t:t + 1])
            nc.vector.tensor_scalar_mul(w[:], w[:], r4[:, t:t + 1])
            wt_new.append(w)
        return wt_new

    def iter_body(h, hp_override=None, extra=None):
        """Emit one pipeline iteration.

        h: head whose scores/softmax run this iteration (None past end).
        hp_override: stage_b head for drain iterations.
        extra: additional scores head emitted after this iteration's PV.
        """
        if h is not None:
            hp = h - 1
        elif hp_override is not None:
            hp = hp_override
        else:
            hp = H - 1
        do_b = hp >= 0 and iter_body.prev is not None
        if do_b:
            wt_list = iter_body.prev
            gb = hp // (H // KVH)
            wT_list = []
        cur = None
        if h is not None:
            g = h // (H // KVH)
            lsum4 = sm.tile([128, NT], f32, tag="l4", name=f"l4_{h}",
                            bufs=2)
            r4 = sm.tile([128, NT], f32, tag="r4", name=f"r4_{h}", bufs=2)
            wt_new = []

        if extra is not None:
            # q-projection of the extra head up front: its PE work leads
            # the iteration and the DVE copy lands before ACT finishes
            # this head's exps, so stage_a(extra) below never stalls
            qproj(extra, dve_copy=True)

        # interleave: transposes(hp, t) with scores(h, t). On the last
        # scores head, emit scores before transposes so the softmax
        # chain of h=15 starts early (shortens the pipeline drain).
        order = ("b", "a") if h != H - 1 else ("a", "b")
        for t in range(NT):
          # qproj between scores t1 and t2: its PE work covers the
          # ps_s buffer-recycle wait (scores t2 needs exp t0 done)
          if t == 2 and h is not None and h + 1 < H and h + 1 != extra:
              qproj(h + 1)
          for phase in order:
            if phase == "b" and do_b:
                pt = ps_pt.tile([128, BAND], bf16, tag="pt",
                                name=f"pt{hp}_{t}")
                for b in range(5):
                    nc.tensor.transpose(
                        pt[:, b * 128:(b + 1) * 128],
                        wt_list[t][:, b * 128:(b + 1) * 128],
                        identb[:])
                wT = sm.tile([128, BAND], bf16, tag="wT",
                             name=f"wT{hp}_{t}", bufs=3)
                if h is None and t % 2 == 1:
                    # drain: ACT is exp-free, split copies across engines
                    nc.scalar.copy(wT[:], pt[:])
                else:
                    nc.vector.tensor_copy(wT[:], pt[:])
                wT_list.append(wT)
                if h is None:
                    # drain iteration: no scores/qproj work to hide the
                    # copy latency, so run each PV right after its tile
                    if t == 0:
                        po_d = ps_ot.tile([128, TLOC], f32, tag="ot",
                                          name=f"po{hp}")
                    for b in range(5):
                        mm(po_d[:, t * 128:(t + 1) * 128],
                           vv[:, t + b, gb * 128:(gb + 1) * 128],
                           wT[:, b * 128:(b + 1) * 128],
                           start=(b == 0), stop=(b == 4))
                    if t == NT - 1:
                        nc.scalar.copy(oT[:, hp, :], po_d[:])
            if phase == "a" and h is not None:
                ps = ps_s.tile([128, 1024], f32, tag="score",
                               name=f"s{h}_{t}")
                mm(ps[:, 0:512], qT[:, h, t * 128:(t + 1) * 128],
                   kT[:, g, t * 128:t * 128 + 512], start=True, stop=True)
                mm(ps[:, 512:BAND], qT[:, h, t * 128:(t + 1) * 128],
                   kT[:, g, t * 128 + 512:t * 128 + BAND], start=True,
                   stop=True)
                w = sm.tile([128, BAND], bf16, tag="w", name=f"w{h}_{t}",
                            bufs=14)
                nc.scalar.activation(w[:], ps[:, 0:BAND],
                                     mybir.ActivationFunctionType.Exp)
                nc.vector.scalar_tensor_tensor(
                    w[:], w[:], 1.0, mask_s[:, t, :],
                    op0=mybir.AluOpType.mult, op1=mybir.AluOpType.mult,
                    accum_out=lsum4[:, t:t + 1])
                # per-tile recip + in-place normalize so the dependency
                # chain drains per-t instead of after all four tiles
                nc.vector.reciprocal(r4[:, t:t + 1], lsum4[:, t:t + 1])
                nc.vector.tensor_scalar_mul(w[:], w[:], r4[:, t:t + 1])
                wt_new.append(w)

        if do_b and h is not None:
            po = ps_ot.tile([128, TLOC], f32, tag="ot", name=f"po{hp}")
            for t in range(NT):
                for b in range(5):
                    mm(po[:, t * 128:(t + 1) * 128],
                       vv[:, t + b, gb * 128:(gb + 1) * 128],
                       wT_list[t][:, b * 128:(b + 1) * 128],
                       start=(b == 0), stop=(b == 4))
            nc.scalar.copy(oT[:, hp, :], po[:])

        if extra is not None:
            # softmax of the extra head runs here, a full iteration
            # before its stage_b, so the final b-only iterations find
            # everything ready (no pipeline-drain stalls)
            iter_body.extra_prev = stage_a(extra)

        if h is not None:
            cur = wt_new
        iter_body.prev = cur

    iter_body.prev = None
    iter_body.extra_prev = None
    qproj(0)
    if True:  # simple pipeline: scores h / stage_b h-1, single drain
        for h in range(H):
            iter_body(h)
        # early-P3 fill: open the dblk-0 output-projection groups for
        # heads 0..14 in the now-idle ps_s banks. This PE work fills the
        # pipeline-drain stalls (the final head's softmax chain and wT
        # copies); head 15 + stop land in P3 proper. Same accumulation
        # order as before -> bit-identical results.
        py_a = ps_s.tile([128, 1024], f32, tag="score", name="py_a")
        py_b = ps_s.tile([128, 1024], f32, tag="score", name="py_b")
        py0 = [py_a[:, 0:512], py_a[:, 512:1024],
               py_b[:, 0:512], py_b[:, 512:1024]]
        for t in range(NT):
            for h in range(H - 1):
                mm(py0[t], oT[:, h, t * 128:(t + 1) * 128],
                   woc0[:, h, :], start=(h == 0), stop=False)
        iter_body(None, hp_override=H - 1)
    else:  # pulled-forward variant (kept for reference; slightly slower)
        for h in range(H - 1):
            iter_body(h, extra=(H - 1 if h == H - 2 else None))
        iter_body(None, hp_override=H - 2)
        iter_body.prev = iter_body.extra_prev
        iter_body(None, hp_override=H - 1)

    sm.release()
    xtp.release()

    # ---------------- P3: output projection ----------------
    ych_p = tc.alloc_tile_pool(name="ych_p", bufs=4, side="right")
    for dblk in range(4):
        if dblk == 0:
            woc = woc0
        else:
            woc = wop.tile([128, H, 512], bf16, tag="wo")
            nc.sync.dma_start(
                woc[:],
                wo_d.ap()[:, dblk * 512:(dblk + 1) * 512]
                .rearrange("(h p) e -> p h e", p=128))
        for t in range(NT):
            if dblk == 0:
                py = py0[t]
                mm(py, oT[:, H - 1, t * 128:(t + 1) * 128],
                   woc[:, H - 1, :], start=False, stop=True)
            else:
                py = ps_ot.tile([128, 512], f32, tag="ot",
                                name=f"py{dblk}_{t}")
                for h in range(H):
                    mm(py[:], oT[:, h, t * 128:(t + 1) * 128],
                       woc[:, h, :], start=(h == 0), stop=(h == H - 1))
            ych = ych_p.tile([128, 512], bf16, tag="ych")
            if t % 2 == 0:
                nc.vector.tensor_copy(ych[:], py if dblk == 0 else py[:])
            else:
                nc.scalar.copy(ych[:], py if dblk == 0 else py[:])
            nc.sync.dma_start(
                y_d.ap()[t * 128:(t + 1) * 128,
                         dblk * 512:(dblk + 1) * 512],
                ych[:])

    ych_p.release()
    wop.release()
    wp.release()
    proj.release()

    if lp is not None:
        lp.__exit__(None, None, None)
        dtile = pers.tile([128, 128], f32, tag="dtile")
        nc.vector.memset(dtile[:], 0.0)
        nc.sync.dma_start(dummy_d.ap(), dtile[0:1, :])

    ps_ot.release()
    ps_pt.release()
    ps_s.release()
    pers.release()


def build_nc(loop_n=None):
    key = ("nc", loop_n)
    if key in _CACHE:
        return _CACHE[key]
    import concourse.bacc as bacc
    import concourse.mybir as mybir
    import concourse.tile as tile
    from concourse.masks import make_identity

    nc = bacc.Bacc("TRN2", target_bir_lowering=False, debug=False,
                   num_devices=N_CORES)
    with tile.TileContext(nc) as tc:
        _emit(nc, tc, tile, mybir, make_identity, loop_n=loop_n)
    nc.compile()
    _CACHE[key] = nc
    return nc


def _bf16(a):
    import ml_dtypes
    return np.asarray(a, np.float32).astype(ml_dtypes.bfloat16)


def make_inputs_for_core(c, xf, Wq, Wk, Wv, Wo):
    """xf: [T, D] float32 (already squeezed)."""
    if c == 0:
        x_c = np.concatenate(
            [np.zeros((WINDOW, D), np.float32), xf[:TLOC]], axis=0)
    else:
        x_c = xf[TLOC * c - WINDOW: TLOC * c + TLOC]
    # host-side transpose: xt[(dc p), j] = x_c[j, dc*128+p]
    xt = np.ascontiguousarray(x_c.T)

    jj = np.arange(BAND)[None, None, :]
    p = np.arange(128)[None, :, None]
    t = np.arange(NT)[:, None, None]
    allowed = (jj >= p) & (jj <= p + WINDOW)
    if c == 0:
        allowed = allowed & (128 * t + jj >= WINDOW)
    allowed = np.broadcast_to(allowed, (NT, 128, BAND))
    mask = np.where(allowed, np.float32(1.0), np.float32(0.0))

    return {
        "xt": _bf16(xt),
        "wq": _bf16(np.asarray(Wq, np.float32) * np.float32(SCALE)),
        "wk": _bf16(Wk),
        "wv": _bf16(Wv),
        "wo": _bf16(Wo),
        "mask": _bf16(mask),
    }


def kernel(x, Wq, Wk, Wv, Wo):
    from concourse.bass_utils import run_bass_kernel_spmd

    nc = build_nc()
    xf = np.asarray(x, np.float32).reshape(T, D)
    in_maps = [make_inputs_for_core(c, xf, Wq, Wk, Wv, Wo)
               for c in range(N_CORES)]
    res = run_bass_kernel_spmd(nc, in_maps, core_ids=list(range(N_CORES)))
    y = np.concatenate(
        [res.results[c]["y"].astype(np.float32) for c in range(N_CORES)],
        axis=0)
    return y.reshape(1, T, D)


# revision 6
# speedup vs baseline: 1.0296x; 1.0030x over previous
"""Trainium2 Bass kernel: GQA sliding-window attention (bf16 redesign).

Problem: B=1, T=4096, D=2048, H=16 q-heads, KVH=4 kv-heads, HD=128,
causal sliding window 512.

Sharding: 8-way sequence parallel. Core c owns query rows
[512c, 512c+512). It receives xT columns for rows [512(c-1), 512(c+1))
(halo of 512 rows; core 0's halo is zeros and is masked out). Weights
replicated. Outputs are disjoint row blocks -> concatenation.

All compute in bf16 (f32 PSUM accumulation). Host pre-transposes x
(so no on-device transpose phase), pre-scales Wq by SCALE, and casts
everything to bf16.

Per-core layouts (SBUF partition dim first):
  xT  [128, 16, 1024] bf16 : xT[p, dc, j] = x[j, 128*dc+p]
  kT  [128, 4, 1024]  bf16 : kT[p, g, j]  = k[j, 128*g+p]
  vv  [128, 8, 512]   bf16 : vv[p, jc, e] = v[128*jc+p, e]
  qT  [128, 16, 512]  bf16 : qT[p, h, i]  = q[i, 128*h+p] (SCALE folded
    into Wq host-side)
  Scores per (h, t): s[i', jj], key j = 128*t + jj, jj in [0,640).
  Softmax without max-subtraction (|scores| < ~6 for this input
    distribution, verified host-side in the test harness).
  Multiplicative {0,1} bf16 mask post-exp, fused with the row-sum
    (scalar_tensor_tensor accum_out).
  Normalization: w *= 1/rowsum on DVE (bf16 4x mode) before the PE
    transpose. (The PE transpose rhs must be a permutation matrix, so
    normalization cannot fold into it.)
  PV per (h, t): 5 chunk matmuls over the 640-key band into a
    [128, 128] window of the per-head po accumulator.
  oT  [128, 16, 512] bf16 ; y = oT.T @ Wo streamed in 512-col blocks.

P1 is paced against the xT DMA chunks: k-projections for g0/g1/g2 keep
6 accumulation groups open (4 ps_s half-windows + 2 ps_ot) and consume
each 4-dc xT chunk as it lands; g3 + v run afterwards from SBUF.
"""

import numpy as np

T = 4096
D = 2048
H = 16
KVH = 4
HD = 128
WINDOW = 512
SCALE = HD ** -0.5
N_CORES = 8
TLOC = T // N_CORES          # 512 own query rows / core
XROWS = TLOC + WINDOW        # 1024 x rows / core (halo + own)
NT = TLOC // 128             # 4 q-tiles of 128 rows
NJC = XROWS // 128           # 8 key chunks of 128
BAND = WINDOW + 128          # 640 key columns per q-tile
DC = D // 128                # 16 d-chunks

_CACHE = {}


def _emit(nc, tc, tile, mybir, make_identity, loop_n=None):
    f32 = mybir.dt.float32
    bf16 = mybir.dt.bfloat16

    timing = loop_n is not None
    kin = "Internal" if timing else "ExternalInput"
    kout = "Internal" if timing else "ExternalOutput"
    # xt: x transposed host-side, [(dc p), j] = x[j, 128*dc+p]
    xt_d = nc.dram_tensor("xt", [D, XROWS], bf16, kind=kin)
    wq_d = nc.dram_tensor("wq", [D, H * HD], bf16, kind=kin)
    wk_d = nc.dram_tensor("wk", [D, KVH * HD], bf16, kind=kin)
    wv_d = nc.dram_tensor("wv", [D, KVH * HD], bf16, kind=kin)
    wo_d = nc.dram_tensor("wo", [H * HD, D], bf16, kind=kin)
    mask_d = nc.dram_tensor("mask", [NT, 128, BAND], bf16, kind=kin)
    y_d = nc.dram_tensor("y", [TLOC, D], bf16, kind=kout)
    if timing:
        dummy_d = nc.dram_tensor("bench_done", [1, 128], f32,
                                 kind="ExternalOutput")

    def mm(out, lhsT, rhs, start, stop):
        nc.tensor.matmul(out, lhsT, rhs, start=start, stop=stop)

    # --- persistent pools (outside timing loop) ---
    # PSUM budget (8 banks): ps_s = 2 bufs x [128,1024]f32 (2 banks each)
    # = 4 banks; ps_pt = 2 bufs x [128,640]bf16 (1 bank each) = 2 banks;
    # ps_ot = 2 bufs x [128,512]f32 = 2 banks.
    pers = tc.alloc_tile_pool(name="pers", bufs=1)
    ps_s = tc.alloc_tile_pool(name="ps_s", bufs=2, space="PSUM")
    ps_pt = tc.alloc_tile_pool(name="ps_pt", bufs=2, space="PSUM")
    ps_ot = tc.alloc_tile_pool(name="ps_ot", bufs=2, space="PSUM")

    identb = pers.tile([128, 128], bf16, tag="identb")
    make_identity(nc, identb[:])

    lp = tc.For_i(0, loop_n, 1) if timing else None
    if lp is not None:
        lp.__enter__()

    proj = tc.alloc_tile_pool(name="proj", bufs=1)
    mask_s = proj.tile([128, NT, BAND], bf16, tag="mask")
    qT = proj.tile([128, H, TLOC], bf16, tag="qT")
    kT = proj.tile([128, KVH, XROWS], bf16, tag="kT")
    vv = proj.tile([128, NJC, KVH * HD], bf16, tag="vv")
    oT = proj.tile([128, H, TLOC], bf16, tag="oT")
    xtp = tc.alloc_tile_pool(name="xtp", bufs=1)
    xT = xtp.tile([128, DC, XROWS], bf16, tag="xT")

    # weight pools (right side; kvw above wp/wop so kvw frees after P1).
    # 2 rotating bufs each: the in-order sync DMA queue stalls on the
    # 3rd wq chunk until qproj(3) frees its buffer, which still lands
    # far ahead of its consumer.
    wp = tc.alloc_tile_pool(name="wp", bufs=2, side="right")
    wop = tc.alloc_tile_pool(name="wop", bufs=2, side="right")
    kvw = tc.alloc_tile_pool(name="kvw", bufs=1, side="right")

    # ---------------- DMA issue (sync queue, in need-order) -------------
    # Order paces P1: wk(g0,g1) -> xt chunks interleaved with wk(g2,g3).
    wk_s = kvw.tile([128, DC, KVH * HD], bf16, tag="wk")
    wv_s = kvw.tile([128, DC, KVH * HD], bf16, tag="wv")
    # Small first pieces so PE starts ~5us in: wk rows for the first 4
    # dc-chunks only (g0/g1 cols), then the first 2-dc xT chunk; the
    # rest of wk lands while those are consumed.
    nc.sync.dma_start(
        wk_s[:, 0:4, 0:256],
        wk_d.ap()[0:512, 0:256].rearrange("(c p) e -> p c e", p=128))
    for lo, hi in ((0, 2), (2, 4), (4, 8), (8, 12), (12, 16)):
        nc.sync.dma_start(
            xT[:, lo:hi, :],
            xt_d.ap()[lo * 128:hi * 128, :]
            .rearrange("(c p) j -> p c j", p=128))
        if hi == 2:
            nc.sync.dma_start(
                wk_s[:, 0:4, 256:512],
                wk_d.ap()[0:512, 256:512]
                .rearrange("(c p) e -> p c e", p=128))
        elif hi == 4:
            nc.sync.dma_start(
                wk_s[:, 4:8, :],
                wk_d.ap()[512:1024, :].rearrange("(c p) e -> p c e", p=128))
        elif hi == 8:
            nc.sync.dma_start(
                wk_s[:, 8:16, :],
                wk_d.ap()[1024:2048, :]
                .rearrange("(c p) e -> p c e", p=128))
    nc.sync.dma_start(
        wv_s[:], wv_d.ap().rearrange("(c p) e -> p c e", p=128))
    wq_c = []
    for cq in range(4):
        wqc = wp.tile([128, DC, 512], bf16, tag="wq", name=f"wq{cq}")
        nc.sync.dma_start(
            wqc[:],
            wq_d.ap()[:, cq * 512:(cq + 1) * 512]
            .rearrange("(c p) e -> p c e", p=128))
        wq_c.append(wqc)
        if cq == 0:
            nc.sync.dma_start(mask_s[:],
                              mask_d.ap().rearrange("t p j -> p t j"))
    # wo block 0 prefetched here: its dblk-0 accumulation groups start
    # right after the last scores iteration (see early-P3 fill below)
    woc0 = wop.tile([128, H, 512], bf16, tag="wo", name="wo0")
    nc.sync.dma_start(
        woc0[:], wo_d.ap()[:, 0:512].rearrange("(h p) e -> p h e", p=128))

    # ---------------- P1: k/v projections ----------------
    # P1a: k for g0..g2, 6 open accumulation groups, paced by xT chunks.
    sA = ps_s.tile([128, 1024], f32, tag="score", name="p1_sA")
    sB = ps_s.tile([128, 1024], f32, tag="score", name="p1_sB")
    oA = ps_ot.tile([128, 512], f32, tag="ot", name="p1_oA")
    oB = ps_ot.tile([128, 512], f32, tag="ot", name="p1_oB")
    kacc = {  # (g, half) -> psum window
        (0, 0): sA[:, 0:512], (0, 1): sA[:, 512:1024],
        (1, 0): sB[:, 0:512], (1, 1): sB[:, 512:1024],
        (2, 0): oA[:], (2, 1): oB[:],
    }
    stages = [((0, 2), (0, 1, 2)), ((2, 4), (0, 1, 2)),
              ((4, 8), (0, 1, 2)), ((8, 12), (0, 1, 2)),
              ((12, 16), (0, 1, 2))]
    for (lo, hi), gs in stages:
        for g in gs:
            for half in range(2):
                for dc in range(lo, hi):
                    mm(kacc[(g, half)],
                       wk_s[:, dc, g * 128:(g + 1) * 128],
                       xT[:, dc, half * 512:(half + 1) * 512],
                       start=(dc == 0), stop=(dc == DC - 1))
    for i, ((g, half), acc) in enumerate(kacc.items()):
        if i % 2 == 0:
            nc.vector.tensor_copy(
                kT[:, g, half * 512:(half + 1) * 512], acc)
        else:
            nc.scalar.copy(kT[:, g, half * 512:(half + 1) * 512], acc)

    # P1b: k g3 + v, xT fully resident.
    for half in range(2):
        pk = ps_ot.tile([128, 512], f32, tag="ot", name=f"pk3_{half}")
        for dc in range(DC):
            mm(pk[:], wk_s[:, dc, 384:512],
               xT[:, dc, half * 512:(half + 1) * 512],
               start=(dc == 0), stop=(dc == DC - 1))
        if half == 0:
            nc.vector.tensor_copy(kT[:, 3, 0:512], pk[:])
        else:
            nc.scalar.copy(kT[:, 3, 512:1024], pk[:])

    for jc in range(NJC):
        pv = ps_ot.tile([128, 512], f32, tag="ot", name=f"pv{jc}")
        for dc in range(DC):
            mm(pv[:], xT[:, dc, jc * 128:(jc + 1) * 128], wv_s[:, dc, :],
               start=(dc == 0), stop=(dc == DC - 1))
        if jc % 2 == 0:
            nc.vector.tensor_copy(vv[:, jc, :], pv[:])
        else:
            nc.scalar.copy(vv[:, jc, :], pv[:])

    kvw.release()

    # ---------------- P2: attention, q projection interleaved ----------
    sm = tc.alloc_tile_pool(name="sm", bufs=2, side="right")

    def qproj(h, dve_copy=False):
        """q projection for head h (runs one head ahead of scores)."""
        wqc = wq_c[h // 4]
        e0 = (h % 4) * 128
        pq = ps_ot.tile([128, TLOC], f32, tag="ot", name=f"pq{h}")
        for dc in range(DC):
            mm(pq[:], wqc[:, dc, e0:e0 + 128], xT[:, dc, WINDOW:XROWS],
               start=(dc == 0), stop=(dc == DC - 1))
        if dve_copy:
            nc.vector.tensor_copy(qT[:, h, :], pq[:])
        else:
            nc.scalar.copy(qT[:, h, :], pq[:])

    def stage_a(h):
        """scores + softmax for head h (qT already resident), with the
        mask+rowsum on the otherwise-idle Pool engine. Used only for the
        pulled-forward last head, whose results are not needed for a
        full iteration (so the slower gpsimd stt is off anyone's
        critical path and DVE stays free for the main head's softmax)."""
        g = h // (H // KVH)
        lsum4 = sm.tile([128, NT], f32, tag="l4", name=f"l4_{h}", bufs=2)
        r4 = sm.tile([128, NT], f32, tag="r4", name=f"r4_{h}", bufs=2)
        wt_new = []
        for t in range(NT):
            ps = ps_s.tile([128, 1024], f32, tag="score", name=f"s{h}_{t}")
            mm(ps[:, 0:512], qT[:, h, t * 128:(t + 1) * 128],
               kT[:, g, t * 128:t * 128 + 512], start=True, stop=True)
            mm(ps[:, 512:BAND], qT[:, h, t * 128:(t + 1) * 128],
               kT[:, g, t * 128 + 512:t * 128 + BAND], start=True,
               stop=True)
            w = sm.tile([128, BAND], bf16, tag="w", name=f"w{h}_{t}",
                        bufs=14)
            nc.scalar.activation(w[:], ps[:, 0:BAND],
                                 mybir.ActivationFunctionType.Exp)
            nc.gpsimd.scalar_tensor_tensor(
                w[:], w[:], 1.0, mask_s[:, t, :],
                op0=mybir.AluOpType.mult, op1=mybir.AluOpType.mult,
                accum_out=lsum4[:, t:t + 1])
            nc.vector.reciprocal(r4[:, t:t + 1], lsum4[:, t:t + 1])
            nc.vector.tensor_scalar_mul(w[:], w[:], r4[:, t:t + 1])
            wt_new.append(w)
        return wt_new

    def iter_body(h, hp_override=None, extra=None):
        """Emit one pipeline iteration.

        h: head whose scores/softmax run this iteration (None past end).
        hp_override: stage_b head for drain iterations.
        extra: additional scores head emitted after this iteration's PV.
        """
        if h is not None:
            hp = h - 1
        elif hp_override is not None:
            hp = hp_override
        else:
            hp = H - 1
        do_b = hp >= 0 and iter_body.prev is not None
        if do_b:
            wt_list = iter_body.prev
            gb = hp // (H // KVH)
            wT_list = []
        cur = None
        if h is not None:
            g = h // (H // KVH)
            lsum4 = sm.tile([128, NT], f32, tag="l4", name=f"l4_{h}",
                            bufs=2)
            r4 = sm.tile([128, NT], f32, tag="r4", name=f"r4_{h}", bufs=2)
            wt_new = []

        if extra is not None:
            # q-projection of the extra head up front: its PE work leads
            # the iteration and the DVE copy lands before ACT finishes
            # this head's exps, so stage_a(extra) below never stalls
            qproj(extra, dve_copy=True)

        # interleave: transposes(hp, t) with scores(h, t). On the last
        # scores head, emit scores before transposes so the softmax
        # chain of h=15 starts early (shortens the pipeline drain).
        order = ("b", "a") if h != H - 1 else ("a", "b")
        for t in range(NT):
          # qproj between scores t1 and t2: its PE work covers the
          # ps_s buffer-recycle wait (scores t2 needs exp t0 done)
          if t == 2 and h is not None and h + 1 < H and h + 1 != extra:
              qproj(h + 1)
          for phase in order:
            if phase == "b" and do_b:
                pt = ps_pt.tile([128, BAND], bf16, tag="pt",
                                name=f"pt{hp}_{t}")
                for b in range(5):
                    nc.tensor.transpose(
                        pt[:, b * 128:(b + 1) * 128],
                        wt_list[t][:, b * 128:(b + 1) * 128],
                        identb[:])
                wT = sm.tile([128, BAND], bf16, tag="wT",
                             name=f"wT{hp}_{t}", bufs=3)
                if h is None and t % 2 == 1:
                    # drain: ACT is exp-free, split copies across engines
                    nc.scalar.copy(wT[:], pt[:])
                else:
                    nc.vector.tensor_copy(wT[:], pt[:])
                wT_list.append(wT)
                if h is None:
                    # drain iteration: no scores/qproj work to hide the
                    # copy latency, so run each PV right after its tile
                    if t == 0:
                        po_d = ps_ot.tile([128, TLOC], f32, tag="ot",
                                          name=f"po{hp}")
                    for b in range(5):
                        mm(po_d[:, t * 128:(t + 1) * 128],
                           vv[:, t + b, gb * 128:(gb + 1) * 128],
                           wT[:, b * 128:(b + 1) * 128],
                           start=(b == 0), stop=(b == 4))
                    if t == NT - 1:
                        nc.scalar.copy(oT[:, hp, :], po_d[:])
            if phase == "a" and h is not None:
                ps = ps_s.tile([128, 1024], f32, tag="score",
                               name=f"s{h}_{t}")
                mm(ps[:, 0:512], qT[:, h, t * 128:(t + 1) * 128],
                   kT[:, g, t * 128:t * 128 + 512], start=True, stop=True)
                mm(ps[:, 512:BAND], qT[:, h, t * 128:(t + 1) * 128],
                   kT[:, g, t * 128 + 512:t * 128 + BAND], start=True,
                   stop=True)
                w = sm.tile([128, BAND], bf16, tag="w", name=f"w{h}_{t}",
                            bufs=14)
                nc.scalar.activation(w[:], ps[:, 0:BAND],
                                     mybir.ActivationFunctionType.Exp)
                nc.vector.scalar_tensor_tensor(
                    w[:], w[:], 1.0, mask_s[:, t, :],
                    op0=mybir.AluOpType.mult, op1=mybir.AluOpType.mult,
                    accum_out=lsum4[:, t:t + 1])
                # per-tile recip + in-place normalize so the dependency
                # chain drains per-t instead of after all four tiles
                nc.vector.reciprocal(r4[:, t:t + 1], lsum4[:, t:t + 1])
                nc.vector.tensor_scalar_mul(w[:], w[:], r4[:, t:t + 1])
                wt_new.append(w)

        if do_b and h is not None:
            po = ps_ot.tile([128, TLOC], f32, tag="ot", name=f"po{hp}")
            for t in range(NT):
                for b in range(5):
                    mm(po[:, t * 128:(t + 1) * 128],
                       vv[:, t + b, gb * 128:(gb + 1) * 128],
                       wT_list[t][:, b * 128:(b + 1) * 128],
                       start=(b == 0), stop=(b == 4))
            nc.scalar.copy(oT[:, hp, :], po[:])

        if extra is not None:
            # softmax of the extra head runs here, a full iteration
            # before its stage_b, so the final b-only iterations find
            # everything ready (no pipeline-drain stalls)
            iter_body.extra_prev = stage_a(extra)

        if h is not None:
            cur = wt_new
        iter_body.prev = cur

    iter_body.prev = None
    iter_body.extra_prev = None
    qproj(0)
    if True:  # simple pipeline: scores h / stage_b h-1, single drain
        for h in range(H):
            iter_body(h)
        # early-P3 fill: open the dblk-0 output-projection groups for
        # heads 0..14 in the now-idle ps_s banks. This PE work fills the
        # pipeline-drain stalls (the final head's softmax chain and wT
        # copies); head 15 + stop land in P3 proper. Same accumulation
        # order as before -> bit-identical results.
        py_a = ps_s.tile([128, 1024], f32, tag="score", name="py_a")
        py_b = ps_s.tile([128, 1024], f32, tag="score", name="py_b")
        py0 = [py_a[:, 0:512], py_a[:, 512:1024],
               py_b[:, 0:512], py_b[:, 512:1024]]
        # drain inlined with the early-P3 fill interleaved per tile:
        # each transpose->copy->PV handoff of the final head hides
        # under ~3us of output-projection matmuls
        wt15 = iter_body.prev
        gb15 = (H - 1) // (H // KVH)
        wT15 = []
        po15 = ps_ot.tile([128, TLOC], f32, tag="ot", name="po15")
        for t in range(NT):
            for hh in range(H - 1):
                mm(py0[t], oT[:, hh, t * 128:(t + 1) * 128],
                   woc0[:, hh, :], start=(hh == 0), stop=False)
            pt = ps_pt.tile([128, BAND], bf16, tag="pt", name=f"ptd_{t}")
            for b in range(5):
                nc.tensor.transpose(
                    pt[:, b * 128:(b + 1) * 128],
                    wt15[t][:, b * 128:(b + 1) * 128], identb[:])
            wTd = sm.tile([128, BAND], bf16, tag="wT", name=f"wTd_{t}",
                          bufs=3)
            if t % 2 == 1:
                nc.scalar.copy(wTd[:], pt[:])
            else:
                nc.vector.tensor_copy(wTd[:], pt[:])
            wT15.append(wTd)
            if t >= 1:
                tp = t - 1
                for b in range(5):
                    mm(po15[:, tp * 128:(tp + 1) * 128],
                       vv[:, tp + b, gb15 * 128:(gb15 + 1) * 128],
                       wT15[tp][:, b * 128:(b + 1) * 128],
                       start=(b == 0), stop=(b == 4))
        tp = NT - 1
        for b in range(5):
            mm(po15[:, tp * 128:(tp + 1) * 128],
               vv[:, tp + b, gb15 * 128:(gb15 + 1) * 128],
               wT15[tp][:, b * 128:(b + 1) * 128],
               start=(b == 0), stop=(b == 4))
        nc.scalar.copy(oT[:, H - 1, :], po15[:])
    else:  # pulled-forward variant (kept for reference; slightly slower)
        for h in range(H - 1):
            iter_body(h, extra=(H - 1 if h == H - 2 else None))
        iter_body(None, hp_override=H - 2)
        iter_body.prev = iter_body.extra_prev
        iter_body(None, hp_override=H - 1)

    sm.release()
    xtp.release()

    # ---------------- P3: output projection ----------------
    ych_p = tc.alloc_tile_pool(name="ych_p", bufs=4, side="right")
    for dblk in range(4):
        if dblk == 0:
            woc = woc0
        else:
            woc = wop.tile([128, H, 512], bf16, tag="wo")
            nc.sync.dma_start(
                woc[:],
                wo_d.ap()[:, dblk * 512:(dblk + 1) * 512]
                .rearrange("(h p) e -> p h e", p=128))
        for t in range(NT):
            if dblk == 0:
                py = py0[t]
                mm(py, oT[:, H - 1, t * 128:(t + 1) * 128],
                   woc[:, H - 1, :], start=False, stop=True)
            else:
                py = ps_ot.tile([128, 512], f32, tag="ot",
                                name=f"py{dblk}_{t}")
                for h in range(H):
                    mm(py[:], oT[:, h, t * 128:(t + 1) * 128],
                       woc[:, h, :], start=(h == 0), stop=(h == H - 1))
            ych = ych_p.tile([128, 512], bf16, tag="ych")
            if t % 2 == 0:
                nc.vector.tensor_copy(ych[:], py if dblk == 0 else py[:])
            else:
                nc.scalar.copy(ych[:], py if dblk == 0 else py[:])
            nc.sync.dma_start(
                y_d.ap()[t * 128:(t + 1) * 128,
                         dblk * 512:(dblk + 1) * 512],
                ych[:])

    ych_p.release()
    wop.release()
    wp.release()
    proj.release()

    if lp is not None:
        lp.__exit__(None, None, None)
        dtile = pers.tile([128, 128], f32, tag="dtile")
        nc.vector.memset(dtile[:], 0.0)
        nc.sync.dma_start(dummy_d.ap(), dtile[0:1, :])

    ps_ot.release()
    ps_pt.release()
    ps_s.release()
    pers.release()


def build_nc(loop_n=None):
    key = ("nc", loop_n)
    if key in _CACHE:
        return _CACHE[key]
    import concourse.bacc as bacc
    import concourse.mybir as mybir
    import concourse.tile as tile
    from concourse.masks import make_identity

    nc = bacc.Bacc("TRN2", target_bir_lowering=False, debug=False,
                   num_devices=N_CORES)
    with tile.TileContext(nc) as tc:
        _emit(nc, tc, tile, mybir, make_identity, loop_n=loop_n)
    nc.compile()
    _CACHE[key] = nc
    return nc


def _bf16(a):
    import ml_dtypes
    return np.asarray(a, np.float32).astype(ml_dtypes.bfloat16)


def make_inputs_for_core(c, xf, Wq, Wk, Wv, Wo):
    """xf: [T, D] float32 (already squeezed)."""
    if c == 0:
        x_c = np.concatenate(
            [np.zeros((WINDOW, D), np.float32), xf[:TLOC]], axis=0)
    else:
        x_c = xf[TLOC * c - WINDOW: TLOC * c + TLOC]
    # host-side transpose: xt[(dc p), j] = x_c[j, dc*128+p]
    xt = np.ascontiguousarray(x_c.T)

    jj = np.arange(BAND)[None, None, :]
    p = np.arange(128)[None, :, None]
    t = np.arange(NT)[:, None, None]
    allowed = (jj >= p) & (jj <= p + WINDOW)
    if c == 0:
        allowed = allowed & (128 * t + jj >= WINDOW)
    allowed = np.broadcast_to(allowed, (NT, 128, BAND))
    mask = np.where(allowed, np.float32(1.0), np.float32(0.0))

    return {
        "xt": _bf16(xt),
        "wq": _bf16(np.asarray(Wq, np.float32) * np.float32(SCALE)),
        "wk": _bf16(Wk),
        "wv": _bf16(Wv),
        "wo": _bf16(Wo),
        "mask": _bf16(mask),
    }


def kernel(x, Wq, Wk, Wv, Wo):
    from concourse.bass_utils import run_bass_kernel_spmd

    nc = build_nc()
    xf = np.asarray(x, np.float32).reshape(T, D)
    in_maps = [make_inputs_for_core(c, xf, Wq, Wk, Wv, Wo)
               for c in range(N_CORES)]
    res = run_bass_kernel_spmd(nc, in_maps, core_ids=list(range(N_CORES)))
    y = np.concatenate(
        [res.results[c]["y"].astype(np.float32) for c in range(N_CORES)],
        axis=0)
    return y.reshape(1, T, D)


# revision 8
# speedup vs baseline: 1.0355x; 1.0058x over previous
"""Trainium2 Bass kernel: GQA sliding-window attention (bf16 redesign).

Problem: B=1, T=4096, D=2048, H=16 q-heads, KVH=4 kv-heads, HD=128,
causal sliding window 512.

Sharding: 8-way sequence parallel. Core c owns query rows
[512c, 512c+512). It receives xT columns for rows [512(c-1), 512(c+1))
(halo of 512 rows; core 0's halo is zeros and is masked out). Weights
replicated. Outputs are disjoint row blocks -> concatenation.

All compute in bf16 (f32 PSUM accumulation). Host pre-transposes x
(so no on-device transpose phase), pre-scales Wq by SCALE, and casts
everything to bf16.

Per-core layouts (SBUF partition dim first):
  xT  [128, 16, 1024] bf16 : xT[p, dc, j] = x[j, 128*dc+p]
  kT  [128, 4, 1024]  bf16 : kT[p, g, j]  = k[j, 128*g+p]
  vv  [128, 8, 512]   bf16 : vv[p, jc, e] = v[128*jc+p, e]
  qT  [128, 16, 512]  bf16 : qT[p, h, i]  = q[i, 128*h+p] (SCALE folded
    into Wq host-side)
  Scores per (h, t): s[i', jj], key j = 128*t + jj, jj in [0,640).
  Softmax without max-subtraction (|scores| < ~6 for this input
    distribution, verified host-side in the test harness).
  Multiplicative {0,1} bf16 mask post-exp, fused with the row-sum
    (scalar_tensor_tensor accum_out).
  Normalization: w *= 1/rowsum on DVE (bf16 4x mode) before the PE
    transpose. (The PE transpose rhs must be a permutation matrix, so
    normalization cannot fold into it.)
  PV per (h, t): 5 chunk matmuls over the 640-key band into a
    [128, 128] window of the per-head po accumulator.
  oT  [128, 16, 512] bf16 ; y = oT.T @ Wo streamed in 512-col blocks.

P1 is paced against the xT DMA chunks: k-projections for g0/g1/g2 keep
6 accumulation groups open (4 ps_s half-windows + 2 ps_ot) and consume
each 4-dc xT chunk as it lands; g3 + v run afterwards from SBUF.
"""

import numpy as np

T = 4096
D = 2048
H = 16
KVH = 4
HD = 128
WINDOW = 512
SCALE = HD ** -0.5
N_CORES = 8
TLOC = T // N_CORES          # 512 own query rows / core
XROWS = TLOC + WINDOW        # 1024 x rows / core (halo + own)
NT = TLOC // 128             # 4 q-tiles of 128 rows
NJC = XROWS // 128           # 8 key chunks of 128
BAND = WINDOW + 128          # 640 key columns per q-tile
DC = D // 128                # 16 d-chunks

_CACHE = {}


def _emit(nc, tc, tile, mybir, make_identity, loop_n=None):
    f32 = mybir.dt.float32
    bf16 = mybir.dt.bfloat16

    timing = loop_n is not None
    kin = "Internal" if timing else "ExternalInput"
    kout = "Internal" if timing else "ExternalOutput"
    # xt: x transposed host-side, [(dc p), j] = x[j, 128*dc+p]
    xt_d = nc.dram_tensor("xt", [D, XROWS], bf16, kind=kin)
    wq_d = nc.dram_tensor("wq", [D, H * HD], bf16, kind=kin)
    wk_d = nc.dram_tensor("wk", [D, KVH * HD], bf16, kind=kin)
    wv_d = nc.dram_tensor("wv", [D, KVH * HD], bf16, kind=kin)
    wo_d = nc.dram_tensor("wo", [H * HD, D], bf16, kind=kin)
    mask_d = nc.dram_tensor("mask", [NT, 128, BAND], bf16, kind=kin)
    y_d = nc.dram_tensor("y", [TLOC, D], bf16, kind=kout)
    if timing:
        dummy_d = nc.dram_tensor("bench_done", [1, 128], f32,
                                 kind="ExternalOutput")

    def mm(out, lhsT, rhs, start, stop):
        nc.tensor.matmul(out, lhsT, rhs, start=start, stop=stop)

    # --- persistent pools (outside timing loop) ---
    # PSUM budget (8 banks): ps_s = 2 bufs x [128,1024]f32 (2 banks each)
    # = 4 banks; ps_pt = 2 bufs x [128,640]bf16 (1 bank each) = 2 banks;
    # ps_ot = 2 bufs x [128,512]f32 = 2 banks.
    pers = tc.alloc_tile_pool(name="pers", bufs=1)
    ps_s = tc.alloc_tile_pool(name="ps_s", bufs=2, space="PSUM")
    ps_pt = tc.alloc_tile_pool(name="ps_pt", bufs=2, space="PSUM")
    ps_ot = tc.alloc_tile_pool(name="ps_ot", bufs=2, space="PSUM")

    identb = pers.tile([128, 128], bf16, tag="identb")
    make_identity(nc, identb[:])

    lp = tc.For_i(0, loop_n, 1) if timing else None
    if lp is not None:
        lp.__enter__()

    proj = tc.alloc_tile_pool(name="proj", bufs=1)
    mask_s = proj.tile([128, NT, BAND], bf16, tag="mask")
    qT = proj.tile([128, H, TLOC], bf16, tag="qT")
    kT = proj.tile([128, KVH, XROWS], bf16, tag="kT")
    vv = proj.tile([128, NJC, KVH * HD], bf16, tag="vv")
    oT = proj.tile([128, H, TLOC], bf16, tag="oT")
    xtp = tc.alloc_tile_pool(name="xtp", bufs=1)
    xT = xtp.tile([128, DC, XROWS], bf16, tag="xT")

    # weight pools (right side; kvw above wp/wop so kvw frees after P1).
    # 2 rotating bufs each: the in-order sync DMA queue stalls on the
    # 3rd wq chunk until qproj(3) frees its buffer, which still lands
    # far ahead of its consumer.
    wp = tc.alloc_tile_pool(name="wp", bufs=2, side="right")
    wop = tc.alloc_tile_pool(name="wop", bufs=2, side="right")
    kvw = tc.alloc_tile_pool(name="kvw", bufs=1, side="right")

    # ---------------- DMA issue (sync queue, in need-order) -------------
    # Order paces P1: wk(g0,g1) -> xt chunks interleaved with wk(g2,g3).
    wk_s = kvw.tile([128, DC, KVH * HD], bf16, tag="wk")
    wv_s = kvw.tile([128, DC, KVH * HD], bf16, tag="wv")
    # Small first pieces so PE starts ~5us in: wk rows for the first 4
    # dc-chunks only (g0/g1 cols), then the first 2-dc xT chunk; the
    # rest of wk lands while those are consumed.
    nc.sync.dma_start(
        wk_s[:, 0:4, 0:256],
        wk_d.ap()[0:512, 0:256].rearrange("(c p) e -> p c e", p=128))
    # first chunk split by j-half too: the half-0 k-proj groups only
    # need j 0:512, so PE starts one half-transfer earlier
    nc.sync.dma_start(
        xT[:, 0:2, 0:512],
        xt_d.ap()[0:256, 0:512].rearrange("(c p) j -> p c j", p=128))
    for lo, hi in ((0, 2), (2, 4), (4, 8), (8, 12), (12, 16)):
        if (lo, hi) == (0, 2):
            nc.sync.dma_start(
                xT[:, 0:2, 512:1024],
                xt_d.ap()[0:256, 512:1024]
                .rearrange("(c p) j -> p c j", p=128))
        else:
            nc.sync.dma_start(
                xT[:, lo:hi, :],
                xt_d.ap()[lo * 128:hi * 128, :]
                .rearrange("(c p) j -> p c j", p=128))
        if hi == 2:
            nc.sync.dma_start(
                wk_s[:, 0:4, 256:512],
                wk_d.ap()[0:512, 256:512]
                .rearrange("(c p) e -> p c e", p=128))
        elif hi == 4:
            nc.sync.dma_start(
                wk_s[:, 4:8, :],
                wk_d.ap()[512:1024, :].rearrange("(c p) e -> p c e", p=128))
        elif hi == 8:
            nc.sync.dma_start(
                wk_s[:, 8:16, :],
                wk_d.ap()[1024:2048, :]
                .rearrange("(c p) e -> p c e", p=128))
    nc.sync.dma_start(
        wv_s[:], wv_d.ap().rearrange("(c p) e -> p c e", p=128))
    wq_c = []
    for cq in range(4):
        wqc = wp.tile([128, DC, 512], bf16, tag="wq", name=f"wq{cq}")
        nc.sync.dma_start(
            wqc[:],
            wq_d.ap()[:, cq * 512:(cq + 1) * 512]
            .rearrange("(c p) e -> p c e", p=128))
        wq_c.append(wqc)
        if cq == 0:
            nc.sync.dma_start(mask_s[:],
                              mask_d.ap().rearrange("t p j -> p t j"))
    # wo block 0 prefetched here: its dblk-0 accumulation groups start
    # right after the last scores iteration (see early-P3 fill below)
    woc0 = wop.tile([128, H, 512], bf16, tag="wo", name="wo0")
    nc.sync.dma_start(
        woc0[:], wo_d.ap()[:, 0:512].rearrange("(h p) e -> p h e", p=128))

    # ---------------- P1: k/v projections ----------------
    # P1a: k for g0..g2, 6 open accumulation groups, paced by xT chunks.
    sA = ps_s.tile([128, 1024], f32, tag="score", name="p1_sA")
    sB = ps_s.tile([128, 1024], f32, tag="score", name="p1_sB")
    oA = ps_ot.tile([128, 512], f32, tag="ot", name="p1_oA")
    oB = ps_ot.tile([128, 512], f32, tag="ot", name="p1_oB")
    kacc = {  # (g, half) -> psum window
        (0, 0): sA[:, 0:512], (0, 1): sA[:, 512:1024],
        (1, 0): sB[:, 0:512], (1, 1): sB[:, 512:1024],
        (2, 0): oA[:], (2, 1): oB[:],
    }
    stages = [((0, 2), (0, 1, 2)), ((2, 4), (0, 1, 2)),
              ((4, 8), (0, 1, 2)), ((8, 12), (0, 1, 2)),
              ((12, 16), (0, 1, 2))]
    for si, ((lo, hi), gs) in enumerate(stages):
        # first stage half-major: half-0 work starts on the j-half
        # transfer, half-1 follows when the second half lands
        halves_outer = si == 0
        for half in range(2) if halves_outer else (None,):
            for g in gs:
                for h2 in ((half,) if halves_outer else range(2)):
                    for dc in range(lo, hi):
                        mm(kacc[(g, h2)],
                           wk_s[:, dc, g * 128:(g + 1) * 128],
                           xT[:, dc, h2 * 512:(h2 + 1) * 512],
                           start=(dc == 0), stop=(dc == DC - 1))
    for i, ((g, half), acc) in enumerate(kacc.items()):
        if i % 2 == 0:
            nc.vector.tensor_copy(
                kT[:, g, half * 512:(half + 1) * 512], acc)
        else:
            nc.scalar.copy(kT[:, g, half * 512:(half + 1) * 512], acc)

    # P1b: k g3 + v, xT fully resident.
    for half in range(2):
        pk = ps_ot.tile([128, 512], f32, tag="ot", name=f"pk3_{half}")
        for dc in range(DC):
            mm(pk[:], wk_s[:, dc, 384:512],
               xT[:, dc, half * 512:(half + 1) * 512],
               start=(dc == 0), stop=(dc == DC - 1))
        if half == 0:
            nc.vector.tensor_copy(kT[:, 3, 0:512], pk[:])
        else:
            nc.scalar.copy(kT[:, 3, 512:1024], pk[:])

    for jc in range(NJC):
        pv = ps_ot.tile([128, 512], f32, tag="ot", name=f"pv{jc}")
        for dc in range(DC):
            mm(pv[:], xT[:, dc, jc * 128:(jc + 1) * 128], wv_s[:, dc, :],
               start=(dc == 0), stop=(dc == DC - 1))
        if jc % 2 == 0:
            nc.vector.tensor_copy(vv[:, jc, :], pv[:])
        else:
            nc.scalar.copy(vv[:, jc, :], pv[:])

    kvw.release()

    # ---------------- P2: attention, q projection interleaved ----------
    sm = tc.alloc_tile_pool(name="sm", bufs=2, side="right")

    def qproj(h, dve_copy=False, dc_range=None):
        """q projection for head h (runs one head ahead of scores).

        dc_range splits the accumulation across two emission points
        (same group, same dc order -> identical math); the psum tile is
        stashed on the function between the halves.
        """
        wqc = wq_c[h // 4]
        e0 = (h % 4) * 128
        lo, hi = dc_range if dc_range is not None else (0, DC)
        if lo == 0:
            qproj.pq = ps_ot.tile([128, TLOC], f32, tag="ot",
                                  name=f"pq{h}")
        pq = qproj.pq
        for dc in range(lo, hi):
            mm(pq[:], wqc[:, dc, e0:e0 + 128], xT[:, dc, WINDOW:XROWS],
               start=(dc == 0), stop=(dc == DC - 1))
        if hi == DC:
            if dve_copy:
                nc.vector.tensor_copy(qT[:, h, :], pq[:])
            else:
                nc.scalar.copy(qT[:, h, :], pq[:])

    def stage_a(h):
        """scores + softmax for head h (qT already resident), with the
        mask+rowsum on the otherwise-idle Pool engine. Used only for the
        pulled-forward last head, whose results are not needed for a
        full iteration (so the slower gpsimd stt is off anyone's
        critical path and DVE stays free for the main head's softmax)."""
        g = h // (H // KVH)
        lsum4 = sm.tile([128, NT], f32, tag="l4", name=f"l4_{h}", bufs=2)
        r4 = sm.tile([128, NT], f32, tag="r4", name=f"r4_{h}", bufs=2)
        wt_new = []
        for t in range(NT):
            ps = ps_s.tile([128, 1024], f32, tag="score", name=f"s{h}_{t}")
            mm(ps[:, 0:512], qT[:, h, t * 128:(t + 1) * 128],
               kT[:, g, t * 128:t * 128 + 512], start=True, stop=True)
            mm(ps[:, 512:BAND], qT[:, h, t * 128:(t + 1) * 128],
               kT[:, g, t * 128 + 512:t * 128 + BAND], start=True,
               stop=True)
            w = sm.tile([128, BAND], bf16, tag="w", name=f"w{h}_{t}",
                        bufs=14)
            nc.scalar.activation(w[:], ps[:, 0:BAND],
                                 mybir.ActivationFunctionType.Exp)
            nc.gpsimd.scalar_tensor_tensor(
                w[:], w[:], 1.0, mask_s[:, t, :],
                op0=mybir.AluOpType.mult, op1=mybir.AluOpType.mult,
                accum_out=lsum4[:, t:t + 1])
            nc.vector.reciprocal(r4[:, t:t + 1], lsum4[:, t:t + 1])
            nc.vector.tensor_scalar_mul(w[:], w[:], r4[:, t:t + 1])
            wt_new.append(w)
        return wt_new

    def iter_body(h, hp_override=None, extra=None):
        """Emit one pipeline iteration.

        h: head whose scores/softmax run this iteration (None past end).
        hp_override: stage_b head for drain iterations.
        extra: additional scores head emitted after this iteration's PV.
        """
        if h is not None:
            hp = h - 1
        elif hp_override is not None:
            hp = hp_override
        else:
            hp = H - 1
        do_b = hp >= 0 and iter_body.prev is not None
        if do_b:
            wt_list = iter_body.prev
            gb = hp // (H // KVH)
            wT_list = []
        cur = None
        if h is not None:
            g = h // (H // KVH)
            lsum4 = sm.tile([128, NT], f32, tag="l4", name=f"l4_{h}",
                            bufs=2)
            r4 = sm.tile([128, NT], f32, tag="r4", name=f"r4_{h}", bufs=2)
            wt_new = []

        if extra is not None:
            # q-projection of the extra head up front: its PE work leads
            # the iteration and the DVE copy lands before ACT finishes
            # this head's exps, so stage_a(extra) below never stalls
            qproj(extra, dve_copy=True)

        # interleave: transposes(hp, t) with scores(h, t). On the last
        # scores head, emit scores before transposes so the softmax
        # chain of h=15 starts early (shortens the pipeline drain).
        order = ("b", "a") if h != H - 1 else ("a", "b")
        for t in range(NT):
          # qproj split across the t2/t3 slots: its PE work covers the
          # ps_s buffer-recycle waits (scores t2/t3 need exp t0/t1 done).
          # The last head's qT copy goes to DVE so ACT reaches the final
          # head's exps sooner (shortens iteration 15's scores waits).
          if t in (2, 3) and h is not None and h + 1 < H \
                  and h + 1 != extra:
              qproj(h + 1, dve_copy=(h + 1 == H - 1),
                    dc_range=(0, 8) if t == 2 else (8, 16))
          for phase in order:
            if phase == "b" and do_b:
                pt = ps_pt.tile([128, BAND], bf16, tag="pt",
                                name=f"pt{hp}_{t}")
                for b in range(5):
                    nc.tensor.transpose(
                        pt[:, b * 128:(b + 1) * 128],
                        wt_list[t][:, b * 128:(b + 1) * 128],
                        identb[:])
                wT = sm.tile([128, BAND], bf16, tag="wT",
                             name=f"wT{hp}_{t}", bufs=3)
                if h is None and t % 2 == 1:
                    # drain: ACT is exp-free, split copies across engines
                    nc.scalar.copy(wT[:], pt[:])
                else:
                    nc.vector.tensor_copy(wT[:], pt[:])
                wT_list.append(wT)
                if h is None:
                    # drain iteration: no scores/qproj work to hide the
                    # copy latency, so run each PV right after its tile
                    if t == 0:
                        po_d = ps_ot.tile([128, TLOC], f32, tag="ot",
                                          name=f"po{hp}")
                    for b in range(5):
                        mm(po_d[:, t * 128:(t + 1) * 128],
                           vv[:, t + b, gb * 128:(gb + 1) * 128],
                           wT[:, b * 128:(b + 1) * 128],
                           start=(b == 0), stop=(b == 4))
                    if t == NT - 1:
                        nc.scalar.copy(oT[:, hp, :], po_d[:])
            if phase == "a" and h is not None:
                ps = ps_s.tile([128, 1024], f32, tag="score",
                               name=f"s{h}_{t}")
                mm(ps[:, 0:512], qT[:, h, t * 128:(t + 1) * 128],
                   kT[:, g, t * 128:t * 128 + 512], start=True, stop=True)
                mm(ps[:, 512:BAND], qT[:, h, t * 128:(t + 1) * 128],
                   kT[:, g, t * 128 + 512:t * 128 + BAND], start=True,
                   stop=True)
                w = sm.tile([128, BAND], bf16, tag="w", name=f"w{h}_{t}",
                            bufs=14)
                nc.scalar.activation(w[:], ps[:, 0:BAND],
                                     mybir.ActivationFunctionType.Exp)
                nc.vector.scalar_tensor_tensor(
                    w[:], w[:], 1.0, mask_s[:, t, :],
                    op0=mybir.AluOpType.mult, op1=mybir.AluOpType.mult,
                    accum_out=lsum4[:, t:t + 1])
                # per-tile recip + in-place normalize so the dependency
                # chain drains per-t instead of after all four tiles
                nc.vector.reciprocal(r4[:, t:t + 1], lsum4[:, t:t + 1])
                nc.vector.tensor_scalar_mul(w[:], w[:], r4[:, t:t + 1])
                wt_new.append(w)

        if do_b and h is not None:
            po = ps_ot.tile([128, TLOC], f32, tag="ot", name=f"po{hp}")
            for t in range(NT):
                for b in range(5):
                    mm(po[:, t * 128:(t + 1) * 128],
                       vv[:, t + b, gb * 128:(gb + 1) * 128],
                       wT_list[t][:, b * 128:(b + 1) * 128],
                       start=(b == 0), stop=(b == 4))
            nc.scalar.copy(oT[:, hp, :], po[:])

        if extra is not None:
            # softmax of the extra head runs here, a full iteration
            # before its stage_b, so the final b-only iterations find
            # everything ready (no pipeline-drain stalls)
            iter_body.extra_prev = stage_a(extra)

        if h is not None:
            cur = wt_new
        iter_body.prev = cur

    iter_body.prev = None
    iter_body.extra_prev = None
    qproj(0)
    if True:  # simple pipeline: scores h / stage_b h-1, single drain
        for h in range(H):
            iter_body(h)
        # early-P3 fill: open the dblk-0 output-projection groups for
        # heads 0..14 in the now-idle ps_s banks. This PE work fills the
        # pipeline-drain stalls (the final head's softmax chain and wT
        # copies); head 15 + stop land in P3 proper. Same accumulation
        # order as before -> bit-identical results.
        py_a = ps_s.tile([128, 1024], f32, tag="score", name="py_a")
        py_b = ps_s.tile([128, 1024], f32, tag="score", name="py_b")
        py0 = [py_a[:, 0:512], py_a[:, 512:1024],
               py_b[:, 0:512], py_b[:, 512:1024]]
        # drain inlined with the early-P3 fill interleaved per tile:
        # each transpose->copy->PV handoff of the final head hides
        # under ~3us of output-projection matmuls
        wt15 = iter_body.prev
        gb15 = (H - 1) // (H // KVH)
        wT15 = []
        po15 = ps_ot.tile([128, TLOC], f32, tag="ot", name="po15")
        for t in range(NT):
            for hh in range(H - 1):
                mm(py0[t], oT[:, hh, t * 128:(t + 1) * 128],
                   woc0[:, hh, :], start=(hh == 0), stop=False)
            pt = ps_pt.tile([128, BAND], bf16, tag="pt", name=f"ptd_{t}")
            for b in range(5):
                nc.tensor.transpose(
                    pt[:, b * 128:(b + 1) * 128],
                    wt15[t][:, b * 128:(b + 1) * 128], identb[:])
            wTd = sm.tile([128, BAND], bf16, tag="wT", name=f"wTd_{t}",
                          bufs=3)
            if t % 2 == 1:
                nc.scalar.copy(wTd[:], pt[:])
            else:
                nc.vector.tensor_copy(wTd[:], pt[:])
            wT15.append(wTd)
            if t >= 1:
                tp = t - 1
                for b in range(5):
                    mm(po15[:, tp * 128:(tp + 1) * 128],
                       vv[:, tp + b, gb15 * 128:(gb15 + 1) * 128],
                       wT15[tp][:, b * 128:(b + 1) * 128],
                       start=(b == 0), stop=(b == 4))
        tp = NT - 1
        for b in range(5):
            mm(po15[:, tp * 128:(tp + 1) * 128],
               vv[:, tp + b, gb15 * 128:(gb15 + 1) * 128],
               wT15[tp][:, b * 128:(b + 1) * 128],
               start=(b == 0), stop=(b == 4))
        nc.scalar.copy(oT[:, H - 1, :], po15[:])
    else:  # pulled-forward variant (kept for reference; slightly slower)
        for h in range(H - 1):
            iter_body(h, extra=(H - 1 if h == H - 2 else None))
        iter_body(None, hp_override=H - 2)
        iter_body.prev = iter_body.extra_prev
        iter_body(None, hp_override=H - 1)

    sm.release()
    xtp.release()

    # ---------------- P3: output projection ----------------
    ych_p = tc.alloc_tile_pool(name="ych_p", bufs=4, side="right")
    for dblk in range(4):
        if dblk == 0:
            woc = woc0
        else:
            woc = wop.tile([128, H, 512], bf16, tag="wo")
            nc.sync.dma_start(
                woc[:],
                wo_d.ap()[:, dblk * 512:(dblk + 1) * 512]
                .rearrange("(h p) e -> p h e", p=128))
        for t in range(NT):
            if dblk == 0:
                py = py0[t]
                mm(py, oT[:, H - 1, t * 128:(t + 1) * 128],
                   woc[:, H - 1, :], start=False, stop=True)
            else:
                py = ps_ot.tile([128, 512], f32, tag="ot",
                                name=f"py{dblk}_{t}")
                for h in range(H):
                    mm(py[:], oT[:, h, t * 128:(t + 1) * 128],
                       woc[:, h, :], start=(h == 0), stop=(h == H - 1))
            ych = ych_p.tile([128, 512], bf16, tag="ych")
            if t % 2 == 0:
                nc.vector.tensor_copy(ych[:], py if dblk == 0 else py[:])
            else:
                nc.scalar.copy(ych[:], py if dblk == 0 else py[:])
            nc.sync.dma_start(
                y_d.ap()[t * 128:(t + 1) * 128,
                         dblk * 512:(dblk + 1) * 512],
                ych[:])

    ych_p.release()
    wop.release()
    wp.release()
    proj.release()

    if lp is not None:
        lp.__exit__(None, None, None)
        dtile = pers.tile([128, 128], f32, tag="dtile")
        nc.vector.memset(dtile[:], 0.0)
        nc.sync.dma_start(dummy_d.ap(), dtile[0:1, :])

    ps_ot.release()
    ps_pt.release()
    ps_s.release()
    pers.release()


def build_nc(loop_n=None):
    key = ("nc", loop_n)
    if key in _CACHE:
        return _CACHE[key]
    import concourse.bacc as bacc
    import concourse.mybir as mybir
    import concourse.tile as tile
    from concourse.masks import make_identity

    nc = bacc.Bacc("TRN2", target_bir_lowering=False, debug=False,
                   num_devices=N_CORES)
    with tile.TileContext(nc) as tc:
        _emit(nc, tc, tile, mybir, make_identity, loop_n=loop_n)
    nc.compile()
    _CACHE[key] = nc
    return nc


def _bf16(a):
    import ml_dtypes
    return np.asarray(a, np.float32).astype(ml_dtypes.bfloat16)


def make_inputs_for_core(c, xf, Wq, Wk, Wv, Wo):
    """xf: [T, D] float32 (already squeezed)."""
    if c == 0:
        x_c = np.concatenate(
            [np.zeros((WINDOW, D), np.float32), xf[:TLOC]], axis=0)
    else:
        x_c = xf[TLOC * c - WINDOW: TLOC * c + TLOC]
    # host-side transpose: xt[(dc p), j] = x_c[j, dc*128+p]
    xt = np.ascontiguousarray(x_c.T)

    jj = np.arange(BAND)[None, None, :]
    p = np.arange(128)[None, :, None]
    t = np.arange(NT)[:, None, None]
    allowed = (jj >= p) & (jj <= p + WINDOW)
    if c == 0:
        allowed = allowed & (128 * t + jj >= WINDOW)
    allowed = np.broadcast_to(allowed, (NT, 128, BAND))
    mask = np.where(allowed, np.float32(1.0), np.float32(0.0))

    return {
        "xt": _bf16(xt),
        "wq": _bf16(np.asarray(Wq, np.float32) * np.float32(SCALE)),
        "wk": _bf16(Wk),
        "wv": _bf16(Wv),
        "wo": _bf16(Wo),
        "mask": _bf16(mask),
    }


def kernel(x, Wq, Wk, Wv, Wo):
    from concourse.bass_utils import run_bass_kernel_spmd

    nc = build_nc()
    xf = np.asarray(x, np.float32).reshape(T, D)
    in_maps = [make_inputs_for_core(c, xf, Wq, Wk, Wv, Wo)
               for c in range(N_CORES)]
    res = run_bass_kernel_spmd(nc, in_maps, core_ids=list(range(N_CORES)))
    y = np.concatenate(
        [res.results[c]["y"].astype(np.float32) for c in range(N_CORES)],
        axis=0)
    return y.reshape(1, T, D)
